# revision 13
# baseline (speedup 1.0000x reference)
"""Single-head attention (B=4, S=2048, D=E=1024) on 8 trn2 NeuronCores.

Sharding: data-parallel over (batch, q-half) -> 8 shards. Each core gets a
1024-row q shard plus the full 2048 keys of its batch; K/V projections are
recomputed on both cores of a batch pair (25% extra flops, zero collectives).

All matmul operands are bf16 (host-cast); PSUM accumulation stays fp32, so
per-value RMS error ~0.1% -- far inside the 2e-2 gate. bf16 runs at the same
1 cycle/row PE rate as fp32r but halves DMA + SBUF, which lets every weight
stay resident (no DRAM bounce) and keeps the PE streaming continuously:

  per-core PE work (cycles @2.4GHz):
    vp 131072 + kp 131072 + qp 65536 + logits 131072 + softmax-sum 16384
    + ctx 131072 + out 65536 = 672k cycles = 280.1us ideal

Schedule: vp -> kp -> qp(qb0) -> qb0 kb-loop [logits|exp|sum|ctx-half1, with
qp(qb1) in the spare PSUM bank] -> ctx-half2 -> qb1 kb-loop [with out(qb0) in
the spare bank] -> ctx-half2 -> out(qb1). PSUM never exceeds 8 banks; weights
for each phase are prefetched during the previous phase via sibling pools.
"""

import os
import numpy as np

P = 128
NEG = -1.0e9


def build_nc(D=1024, E=1024, SK=2048, QSH=1024, QB=512):
    """Build the per-core Bass module (SPMD; same program on all cores)."""
    import concourse.bass as bass
    import concourse.mybir as mybir
    import concourse.tile as tile
    from concourse import bacc

    f32 = mybir.dt.float32
    bf16 = mybir.dt.bfloat16
    AF = mybir.ActivationFunctionType

    DT = D // P          # contraction tiles over model dim
    ET = E // P          # enc tiles
    KT = SK // P         # key tiles
    NQB = QSH // QB      # q blocks (2)
    KNB = 512            # key free-dim block for kp
    DNB = 512            # model free-dim block for out
    MQ = QB // P         # q 128-row groups per block (4)
    ND = D // DNB        # out column chunks (2)
    ISCALE = 1.0 / float(np.sqrt(E))

    nc = bacc.Bacc(trn_type="TRN2")

    # ---- I/O (bf16 operands; f32 biases/mask; f32 output) ----
    qT = nc.dram_tensor("qT", [D, QSH], bf16, kind="ExternalInput")[:, :]
    kT = nc.dram_tensor("kT", [D, SK], bf16, kind="ExternalInput")[:, :]
    vT = nc.dram_tensor("vT", [D, SK], bf16, kind="ExternalInput")[:, :]
    mask_cols = nc.dram_tensor("mask_cols", [P, KT], f32, kind="ExternalInput")[:, :]
    ones_d = nc.dram_tensor("ones_d", [P, P], bf16, kind="ExternalInput")[:, :]
    wq = nc.dram_tensor("wq", [D, E], bf16, kind="ExternalInput")[:, :]
    wk = nc.dram_tensor("wk", [D, E], bf16, kind="ExternalInput")[:, :]
    wv = nc.dram_tensor("wv", [D, E], bf16, kind="ExternalInput")[:, :]
    ow = nc.dram_tensor("ow", [E, D], bf16, kind="ExternalInput")[:, :]
    bq_col = nc.dram_tensor("bq_col", [P, ET], f32, kind="ExternalInput")[:, :]
    bk_col = nc.dram_tensor("bk_col", [P, ET], f32, kind="ExternalInput")[:, :]
    bv_bc = nc.dram_tensor("bv_bc", [P, E], bf16, kind="ExternalInput")[:, :]
    ob_bc = nc.dram_tensor("ob_bc", [P, D], f32, kind="ExternalInput")[:, :]
    # delta-row selector + ob with row 0 = out bias: the final out group
    # folds its bias in via matmul so the evac is a plain ACT copy
    ob_sel = nc.dram_tensor("ob_sel", [P, P], bf16, kind="ExternalInput")[:, :]
    ob_mat = nc.dram_tensor("ob_mat", [P, D], bf16, kind="ExternalInput")[:, :]
    out = nc.dram_tensor("out", [QSH, D], f32, kind="ExternalOutput")[:, :]

    qT_r = qT.rearrange("(t p) n -> p t n", p=P)   # [128, DT, QSH]
    kT_r = kT.rearrange("(t p) n -> p t n", p=P)
    vT_r = vT.rearrange("(t p) n -> p t n", p=P)
    wq_r = wq.rearrange("(t p) n -> p t n", p=P)   # [128, DT, E]
    wk_r = wk.rearrange("(t p) n -> p t n", p=P)
    wv_r = wv.rearrange("(t p) n -> p t n", p=P)
    ow_r = ow.rearrange("(t p) n -> p t n", p=P)   # [128, ET, D]

    def mm(ps, lhsT, rhs, start, stop):
        nc.tensor.matmul(ps, lhsT, rhs, start=start, stop=stop)

    NWARM = 10

    with tile.TileContext(nc) as tc:
        # ---- persistent smalls (tiles allocated here; DMAs emitted inside
        # the AB scope so the scalar queue prioritizes wv chunks) ----
        with tc.tile_pool(name="smalls", bufs=1) as smalls:
            bv_t = smalls.tile([P, E], bf16, name="bv_t")
            mask_t = smalls.tile([P, KT], f32, name="maskc")
            bk_t = smalls.tile([P, ET], f32, name="bkc")
            bq_t = smalls.tile([P, ET], f32, name="bqc")
            ones_t = smalls.tile([P, P], bf16, name="ones")

            # persistent operand tensors
            with tc.tile_pool(name="wqp", bufs=1) as wqp, \
                 tc.tile_pool(name="vpp", bufs=1) as vpp, \
                 tc.tile_pool(name="kpp", bufs=1) as kpp, \
                 tc.tile_pool(name="qpp", bufs=1) as qpp, \
                 tc.tile_pool(name="expp", bufs=1) as expp, \
                 tc.tile_pool(name="ctxp", bufs=1) as ctxp:
                wq_t = wqp.tile([P, DT, E], bf16, name="wq_t")
                vp = vpp.tile([P, KT, E], bf16, name="vp")      # [k, E]
                kp = kpp.tile([P, ET, SK], bf16, name="kp")     # [E, k] (kp^T)
                qps = [qpp.tile([P, ET, QB], bf16, name=f"qp{i}")
                       for i in range(NQB)]                      # [E, q] (qp^T)
                expT = expp.tile([P, KT, QB], bf16, name="expT")  # [k, q]
                ctxs = [ctxp.tile([P, ET, QB], bf16, name=f"ctx{i}")
                        for i in range(NQB)]                     # [E, q] (ctx^T)

                # ============ phase A+B: vp then kp (sibling pools so kp
                # weights prefetch during vp) ============
                with tc.tile_pool(name="wv_w", bufs=1) as wvp, \
                     tc.tile_pool(name="wk_w", bufs=1) as wkp, \
                     tc.tile_pool(name="vT_s", bufs=2) as vts, \
                     tc.tile_pool(name="kT_s", bufs=2) as kts, \
                     tc.tile_pool(name="qT0_s", bufs=1) as qt0s, \
                     tc.tile_pool(name="warm", bufs=1) as warm, \
                     tc.tile_pool(name="warm_ps", bufs=1,
                                  space="PSUM") as wps, \
                     tc.tile_pool(name="ab_ps", bufs=3, space="PSUM") as abps:
                    # PE warm-up: dummy matmuls on a memset tile fill the
                    # ~5.7us wait for the first weight DMA and ramp the PE
                    # p-state so real work starts at full clock. Lives in
                    # the AB scope so nothing aliases (and WAR-waits on) it.
                    junk = warm.tile([P, 512], bf16, name="junk")
                    nc.vector.memset(junk[:], 0.0)
                    wp = wps.tile([P, 512], f32, name="warmps")
                    for i in range(NWARM):
                        nc.tensor.matmul(wp[:], junk[:, 0:P], junk[:],
                                         start=(i == 0),
                                         stop=(i == NWARM - 1))
                    # The shared DMA bus serves transfers in descriptor-gen
                    # completion order, so every queue is sequenced by first
                    # NEED: sync gets only the first wv half; Pool (slow 1.3us
                    # SWDGE gen each = natural pacing) carries the whole
                    # vp/kp-phase stream in consumption order; scalar gets the
                    # smalls then the late-needed wq/qt0.
                    wv_t = wvp.tile([P, DT, E], bf16, name="wv_t")
                    wk_t = wkp.tile([P, DT, E], bf16, name="wk_t")
                    nc.sync.dma_start(wv_t[:, 0:4, 0:512], wv_r[:, 0:4, 0:512])
                    nc.scalar.dma_start(wv_t[:, 4:8, 0:512],
                                        wv_r[:, 4:8, 0:512])
                    # smalls on scalar, ordered by first use (both bv halves
                    # ride the Pool queue so they can't delay the vT stream)
                    nc.scalar.dma_start(mask_t[:], mask_cols)
                    nc.scalar.mul(mask_t[:], mask_t[:], NEG)
                    nc.scalar.dma_start(bk_t[:], bk_col)
                    nc.scalar.dma_start(bq_t[:], bq_col)
                    nc.scalar.mul(bq_t[:], bq_t[:], ISCALE)
                    nc.scalar.dma_start(ones_t[:], ones_d)
                    qt0 = qt0s.tile([P, DT, QB], bf16, name="qt0")

                    # -- vp: psum [128k, 512E] per (m, n) group; vT streamed
                    # in 1MB chunks of 4 k-tiles (desc count is per (p,t),
                    # so wider chunks halve Pool SWDGE time) --
                    vtiles = {}

                    def load_vt(c):
                        vt = vts.tile([P, DT, 4 * P], bf16, tag="vt",
                                      name=f"vt{c}")
                        if c == 0:
                            # split so each early k-tile unblocks as it lands
                            nc.gpsimd.dma_start(vt[:, :, 0:P],
                                                vT_r[:, :, 0:P])
                            nc.gpsimd.dma_start(vt[:, :, P:2 * P],
                                                vT_r[:, :, P:2 * P])
                            nc.gpsimd.dma_start(vt[:, :, 2 * P:4 * P],
                                                vT_r[:, :, 2 * P:4 * P])
                        else:
                            nc.gpsimd.dma_start(
                                vt[:], vT_r[:, :, 4 * c * P:(4 * c + 4) * P])
                        vtiles[c] = vt

                    def vp_group(m, n):
                        ps = abps.tile([P, 512], f32, tag="ps",
                                       name=f"vps{m}_{n}")
                        vt = vtiles[m // 4]
                        mi = m % 4
                        for t in range(DT):
                            mm(ps[:], vt[:, t, mi * P:(mi + 1) * P],
                               wv_t[:, t, n * 512:(n + 1) * 512],
                               t == 0, t == DT - 1)
                        nc.vector.tensor_add(
                            vp[:, m, n * 512:(n + 1) * 512], ps[:],
                            bv_t[:, n * 512:(n + 1) * 512])

                    # chunk 0: n=0 groups first (wv col half 1 still loading)
                    load_vt(0)
                    # Pool/SWDGE queue carries everything else in strict
                    # consumption order (scalar/sync are HWDGE-fast and would
                    # let late-needed weights jump the shared bus)
                    nc.gpsimd.dma_start(bv_t[:, 0:512], bv_bc[:, 0:512])
                    nc.gpsimd.dma_start(wv_t[:, :, 512:1024],
                                        wv_r[:, :, 512:1024])
                    nc.gpsimd.dma_start(bv_t[:, 512:1024], bv_bc[:, 512:1024])
                    load_vt(1)
                    for m in range(4):
                        vp_group(m, 0)
                    for m in range(4):
                        vp_group(m, 1)
                    for c in range(1, KT // 4):
                        if c + 1 < KT // 4:
                            load_vt(c + 1)
                        for mi in range(4):
                            for n in range(E // 512):
                                vp_group(4 * c + mi, n)
                    # wk lands on the bus after vt2/vt3 (needed at kp start)
                    nc.gpsimd.dma_start(wk_t[:, :, 0:512], wk_r[:, :, 0:512])
                    nc.gpsimd.dma_start(wk_t[:, :, 512:1024],
                                        wk_r[:, :, 512:1024])

                    # -- kp: for each k-chunk, psum [128E, 512k] x8 --
                    for n in range(SK // KNB):
                        kt = kts.tile([P, DT, KNB], bf16, tag="kt",
                                      name=f"kt{n}")
                        nc.gpsimd.dma_start(kt[:],
                                            kT_r[:, :, n * KNB:(n + 1) * KNB])
                        for m in range(ET):
                            ps = abps.tile([P, KNB], f32, tag="ps",
                                           name=f"kps{n}_{m}")
                            for t in range(DT):
                                mm(ps[:], wk_t[:, t, m * P:(m + 1) * P],
                                   kt[:, t, :], t == 0, t == DT - 1)
                            nc.scalar.activation(
                                kp[:, m, n * KNB:(n + 1) * KNB], ps[:],
                                AF.Identity, bias=bk_t[:, m:m + 1])

                    # wq/qt0 queue behind the kp stream; kt2/kt3's WAR head-
                    # of-line block paces their generation to ~90us, arriving
                    # in time for qp0 at ~116us
                    nc.gpsimd.dma_start(wq_t[:, 0:4, :], wq_r[:, 0:4, :])
                    nc.gpsimd.dma_start(wq_t[:, 4:8, :], wq_r[:, 4:8, :])
                    nc.gpsimd.dma_start(qt0[:], qT_r[:, :, 0:QB])

                    # -- qp(qb0): psum [128E, 512q] x8 (reuse ab psum bufs) --
                    for m in range(ET):
                        ps = abps.tile([P, QB], f32, tag="ps", name=f"qps0_{m}")
                        for t in range(DT):
                            mm(ps[:], wq_t[:, t, m * P:(m + 1) * P],
                               qt0[:, t, :], t == 0, t == DT - 1)
                        nc.scalar.activation(qps[0][:, m, :], ps[:],
                                             AF.Identity,
                                             bias=bq_t[:, m:m + 1],
                                             scale=ISCALE)

                # ============ attention (ow/qT1/out-staging reuse AB space) ==
                with tc.tile_pool(name="ow_w", bufs=1) as owp, \
                     tc.tile_pool(name="qT1_s", bufs=1) as qt1s, \
                     tc.tile_pool(name="obp", bufs=1) as obp, \
                     tc.tile_pool(name="smx", bufs=1) as smx, \
                     tc.tile_pool(name="outsb", bufs=4) as osb:
                    # single recip/acc/accb tiles shared across q-blocks
                    # (their lifetimes don't overlap). acc is the DVE-side
                    # softmax-sum accumulator (f32); accb its bf16 staging
                    # for the final cross-partition ones-matmul: moves the
                    # per-kb sum matmul (16384 cycles) off the PE
                    recip_t = smx.tile([P, QB], f32, name="recip")
                    acc_t = smx.tile([P, QB], f32, name="acc")
                    accb_t = smx.tile([P, QB], bf16, name="accb")
                    # Pool/SWDGE queue again: these generate after the kp
                    # stream + wq/qt0, landing well before first use (~160us+)
                    qt1 = qt1s.tile([P, DT, QB], bf16, name="qt1")
                    nc.gpsimd.dma_start(qt1[:], qT_r[:, :, QB:2 * QB])
                    ow_t = owp.tile([P, ET, D], bf16, name="ow_t")
                    nc.gpsimd.dma_start(ow_t[:, 0:4, :], ow_r[:, 0:4, :])
                    nc.gpsimd.dma_start(ow_t[:, 4:8, :], ow_r[:, 4:8, :])
                    ob_t = obp.tile([P, D], f32, name="ob_t")
                    nc.gpsimd.dma_start(ob_t[:], ob_bc)
                    obsel_t = obp.tile([P, P], bf16, name="obsel_t")
                    nc.gpsimd.dma_start(obsel_t[:], ob_sel)
                    obmat_t = obp.tile([P, D], bf16, name="obmat_t")
                    nc.gpsimd.dma_start(obmat_t[:], ob_mat)

                    def out_group(qb, g, spare_pool, store_eng=None):
                        """out[qb*QB+mq*128 : +128, nd*512 : +512] (8 mm)."""
                        nd, mq = divmod(g, MQ)
                        ps = spare_pool.tile([P, DNB], f32, tag="sp",
                                             name=f"ops{qb}_{g}")
                        for e in range(ET):
                            mm(ps[:], ctxs[qb][:, e, mq * P:(mq + 1) * P],
                               ow_t[:, e, nd * DNB:(nd + 1) * DNB],
                               e == 0, e == ET - 1)
                        r0 = qb * QB + mq * P
                        ot = osb.tile([P, DNB], f32, tag="ot",
                                      name=f"ot{qb}_{g}")
                        nc.vector.tensor_add(
                            ot[:], ps[:], ob_t[:, nd * DNB:(nd + 1) * DNB])
                        (store_eng or nc.gpsimd).dma_start(
                            out[r0:r0 + P, nd * DNB:(nd + 1) * DNB], ot[:])

                    def out_group_tail(qb, g, tail_pool):
                        """Last group: bias folded in via the delta-row
                        matmul, evac by plain ACT copy (no DVE bias add on
                        the critical path), 2 column chunks so chunk 0's
                        store overlaps chunk 1's matmuls; final store rides
                        the fast HWDGE queue."""
                        nd, mq = divmod(g, MQ)
                        r0 = qb * QB + mq * P
                        engs = [nc.gpsimd, nc.sync]
                        widths = [3 * DNB // 4, DNB // 4]
                        for j in range(2):
                            c0 = nd * DNB + j * widths[0]
                            w = widths[j]
                            ps = tail_pool.tile([P, w], f32, tag=f"tp{j}",
                                                name=f"opst{qb}_{g}_{j}")
                            ot = osb.tile([P, w], f32, tag=f"ott{j}",
                                          name=f"ott{qb}_{g}_{j}")
                            mm(ps[:], obsel_t[:], obmat_t[:, c0:c0 + w],
                               True, False)
                            for e in range(ET):
                                mm(ps[:],
                                   ctxs[qb][:, e, mq * P:(mq + 1) * P],
                                   ow_t[:, e, c0:c0 + w],
                                   False, e == ET - 1)
                            nc.scalar.activation(ot[:], ps[:], AF.Identity)
                            engs[j].dma_start(
                                out[r0:r0 + P, c0:c0 + w], ot[:])

                    for qb in range(NQB):
                        # banks 0-3: ctx half1 accumulators (live whole block)
                        cps_cm = tc.tile_pool(name=f"cps{qb}", bufs=1,
                                              space="PSUM")
                        cpsp = cps_cm.__enter__()
                        cps = [cpsp.tile([P, QB], f32, name=f"c{qb}_{e}")
                               for e in range(ET // 2)]
                        # banks 4-7: spare + logits triple-buffer. Open order
                        # matters: the allocator hands the most-recently-freed
                        # banks to the first-opened pool, and the previous
                        # qb's last-freed banks (ctx-half2, evacuated latest)
                        # must NOT land on lg, whose first use is immediate.
                        # spare's first use (kb>=1) absorbs that latency.
                        with tc.tile_pool(name=f"spare{qb}", bufs=1,
                                          space="PSUM") as spp, \
                             tc.tile_pool(name=f"lg{qb}", bufs=3,
                                          space="PSUM") as lgp:

                            def lg_mm(kb):
                                ps = lgp.tile([P, QB], f32, tag="lg",
                                              name=f"lg{qb}_{kb}")
                                for e in range(ET):
                                    mm(ps[:], kp[:, e, kb * P:(kb + 1) * P],
                                       qps[qb][:, e, :], e == 0, e == ET - 1)
                                nc.scalar.activation(
                                    expT[:, kb, :], ps[:], AF.Exp,
                                    bias=mask_t[:, kb:kb + 1])
                                # fold exp tile into the DVE softmax-sum acc
                                if kb == 0:
                                    nc.vector.tensor_copy(acc_t[:],
                                                          expT[:, 0, :])
                                else:
                                    nc.vector.tensor_add(acc_t[:],
                                                         acc_t[:],
                                                         expT[:, kb, :])

                            def tail_mm(kb):
                                for e in range(ET // 2):
                                    mm(cps[e][:],
                                       vp[:, kb, e * P:(e + 1) * P],
                                       expT[:, kb, :], kb == 0, kb == KT - 1)

                            for kb in range(KT):
                                lg_mm(kb)
                                if kb > 0:
                                    tail_mm(kb - 1)
                                # spare-bank work, one group per slot:
                                if qb == 0 and kb >= ET:
                                    # qp(qb1) group m = kb-8
                                    m = kb - ET
                                    ps = spp.tile([P, QB], f32, tag="sp",
                                                  name=f"qps1_{m}")
                                    for t in range(DT):
                                        mm(ps[:], wq_t[:, t, m * P:(m + 1) * P],
                                           qt1[:, t, :], t == 0, t == DT - 1)
                                    nc.scalar.activation(
                                        qps[1][:, m, :], ps[:], AF.Identity,
                                        bias=bq_t[:, m:m + 1], scale=ISCALE)
                                if qb == 1 and kb % 2 == 1:
                                    out_group(0, kb // 2, spp)
                            tail_mm(KT - 1)

                        # ctx half2 on a 2-bank ping-pong: only two banks
                        # inherit late evacuations, so the next qb's logits
                        # pool (first-fit) lands on early-freed banks and
                        # starts without waiting. One more bank carries the
                        # cross-partition ones-matmul of the DVE sum acc.
                        with tc.tile_pool(name=f"sum{qb}", bufs=1,
                                          space="PSUM") as sump, \
                             tc.tile_pool(name=f"cps2_{qb}", bufs=2,
                                          space="PSUM") as cps2p:
                            nc.vector.tensor_copy(accb_t[:], acc_t[:])
                            s_ps = sump.tile([P, QB], f32, name=f"sps{qb}")
                            mm(s_ps[:], ones_t[:], accb_t[:],
                               True, True)
                            nc.vector.reciprocal(recip_t[:], s_ps[:])
                            # evacuate half1 (DVE) while half2 accumulates
                            for e in range(ET // 2):
                                nc.vector.tensor_mul(ctxs[qb][:, e, :],
                                                     cps[e][:],
                                                     recip_t[:])
                            for ei in range(ET // 2):
                                e = ET // 2 + ei
                                c2 = cps2p.tile([P, QB], f32, tag="c2",
                                                name=f"c2_{qb}_{e}")
                                for kb in range(KT):
                                    mm(c2[:],
                                       vp[:, kb, e * P:(e + 1) * P],
                                       expT[:, kb, :], kb == 0, kb == KT - 1)
                                # evac right away so the bank frees for the
                                # next sweep / next qb's pools
                                nc.vector.tensor_mul(ctxs[qb][:, e, :],
                                                     c2[:],
                                                     recip_t[:])
                        cps_cm.__exit__(None, None, None)

                    # ---- final out phase: out(qb1) ----
                    with tc.tile_pool(name="out_ps", bufs=2,
                                      space="PSUM") as ops, \
                         tc.tile_pool(name="tail_ps", bufs=1,
                                      space="PSUM") as tps:
                        # final-phase stores ride the idle HWDGE queues
                        # (scalar/sync) so Pool has nothing to drain at the
                        # kernel tail
                        for g in range(ND * MQ - 1):
                            out_group(1, g, ops,
                                      store_eng=(nc.scalar, nc.sync)[g % 2])
                        out_group_tail(1, ND * MQ - 1, tps)

    nc.compile()
    return nc


def make_in_maps(v, k, q, mask, wq_w, wq_b, wk_w, wk_b, wv_w, wv_b, out_w, out_b,
                 n_cores=8, D=1024, E=1024, SK=2048, QSH=1024):
    """Host-side shard + layout prep (data movement + bf16 cast, no math)."""
    import ml_dtypes
    bf = ml_dtypes.bfloat16
    ET = E // P
    KT = SK // P
    f = np.float32
    wq_w = np.ascontiguousarray(np.asarray(wq_w, f).astype(bf))
    wk_w = np.ascontiguousarray(np.asarray(wk_w, f).astype(bf))
    wv_w = np.ascontiguousarray(np.asarray(wv_w, f).astype(bf))
    out_w = np.ascontiguousarray(np.asarray(out_w, f).astype(bf))
    bq_col = np.ascontiguousarray(np.asarray(wq_b, f).reshape(ET, P).T)
    bk_col = np.ascontiguousarray(np.asarray(wk_b, f).reshape(ET, P).T)
    bv_bc = np.ascontiguousarray(
        np.broadcast_to(np.asarray(wv_b, f).astype(bf), (P, E)))
    ob_bc = np.ascontiguousarray(
        np.broadcast_to(np.asarray(out_b, f), (P, len(out_b))))
    ones_arr = np.ones((P, P), bf)
    ob_sel = np.zeros((P, P), bf)
    ob_sel[0, :] = 1
    ob_mat = np.zeros((P, len(out_b)), bf)
    ob_mat[0, :] = np.asarray(out_b, f).astype(bf)
    in_maps = []
    for c in range(n_cores):
        b, h = divmod(c, 2)
        qTc = np.ascontiguousarray(
            np.asarray(q[b, h * QSH:(h + 1) * QSH, :], f).T.astype(bf))
        kTc = np.ascontiguousarray(np.asarray(k[b], f).T.astype(bf))
        vTc = np.ascontiguousarray(np.asarray(v[b], f).T.astype(bf))
        mc = np.ascontiguousarray(np.asarray(mask[b, 0], f).reshape(KT, P).T)
        in_maps.append(dict(qT=qTc, kT=kTc, vT=vTc, mask_cols=mc,
                            ones_d=ones_arr,
                            wq=wq_w, wk=wk_w, wv=wv_w, ow=out_w,
                            bq_col=bq_col, bk_col=bk_col,
                            bv_bc=bv_bc, ob_bc=ob_bc,
                            ob_sel=ob_sel, ob_mat=ob_mat))
    return in_maps


_NC_CACHE = {}


def kernel(v, k, q, mask, wq_w, wq_b, wk_w, wk_b, wv_w, wv_b, out_w, out_b):
    from concourse.bass_utils import run_bass_kernel_spmd

    B, S, D = 4, 2048, 1024
    E, QSH = 1024, 1024
    if "nc" not in _NC_CACHE:
        _NC_CACHE["nc"] = build_nc(D=D, E=E, SK=S, QSH=QSH, QB=512)
    nc = _NC_CACHE["nc"]

    in_maps = make_in_maps(v, k, q, mask, wq_w, wq_b, wk_w, wk_b, wv_w, wv_b,
                           out_w, out_b, n_cores=8, D=D, E=E, SK=S, QSH=QSH)
    trace = bool(int(os.environ.get("BASS_KERNEL_TRACE", "0")))
    res = run_bass_kernel_spmd(nc, in_maps, core_ids=list(range(8)), trace=trace)
    if trace:
        print(f"HW exec time: {res.exec_time_ns} ns")
        _NC_CACHE["last_exec_time_ns"] = res.exec_time_ns
        _NC_CACHE["last_trace"] = res.instructions_and_trace

    outp = np.empty((B, S, D), np.float32)
    for c in range(8):
        b, h = divmod(c, 2)
        outp[b, h * QSH:(h + 1) * QSH, :] = res.results[c]["out"]
    return outp



# revision 25
# speedup vs baseline: 1.0006x; 1.0006x over previous
"""Single-head attention (B=4, S=2048, D=E=1024) on 8 trn2 NeuronCores.

Sharding: data-parallel over (batch, q-half) -> 8 shards. Each core gets a
1024-row q shard plus the full 2048 keys of its batch; K/V projections are
recomputed on both cores of a batch pair (25% extra flops, zero collectives).

All matmul operands are bf16 (host-cast); PSUM accumulation stays fp32, so
per-value RMS error ~0.1% -- far inside the 2e-2 gate. bf16 runs at the same
1 cycle/row PE rate as fp32r but halves DMA + SBUF, which lets every weight
stay resident (no DRAM bounce) and keeps the PE streaming continuously:

  per-core PE work (cycles @2.4GHz):
    vp 131072 + kp 131072 + qp 65536 + logits 131072 + softmax-sum 16384
    + ctx 131072 + out 65536 = 672k cycles = 280.1us ideal

Schedule: vp -> kp -> qp(qb0) -> qb0 kb-loop [logits|exp|sum|ctx-half1, with
qp(qb1) in the spare PSUM bank] -> ctx-half2 -> qb1 kb-loop [with out(qb0) in
the spare bank] -> ctx-half2 -> out(qb1). PSUM never exceeds 8 banks; weights
for each phase are prefetched during the previous phase via sibling pools.
"""

import os
import numpy as np

P = 128
NEG = -1.0e9


def build_nc(D=1024, E=1024, SK=2048, QSH=1024, QB=512):
    """Build the per-core Bass module (SPMD; same program on all cores)."""
    import concourse.bass as bass
    import concourse.mybir as mybir
    import concourse.tile as tile
    from concourse import bacc

    f32 = mybir.dt.float32
    bf16 = mybir.dt.bfloat16
    AF = mybir.ActivationFunctionType

    DT = D // P          # contraction tiles over model dim
    ET = E // P          # enc tiles
    KT = SK // P         # key tiles
    NQB = QSH // QB      # q blocks (2)
    KNB = 512            # key free-dim block for kp
    DNB = 512            # model free-dim block for out
    MQ = QB // P         # q 128-row groups per block (4)
    ND = D // DNB        # out column chunks (2)
    ISCALE = 1.0 / float(np.sqrt(E))

    nc = bacc.Bacc(trn_type="TRN2")

    # ---- I/O (bf16 operands; f32 biases/mask; f32 output) ----
    qT = nc.dram_tensor("qT", [D, QSH], bf16, kind="ExternalInput")[:, :]
    kT = nc.dram_tensor("kT", [D, SK], bf16, kind="ExternalInput")[:, :]
    vT = nc.dram_tensor("vT", [D, SK], bf16, kind="ExternalInput")[:, :]
    mask_cols = nc.dram_tensor("mask_cols", [P, KT], f32, kind="ExternalInput")[:, :]
    ones_d = nc.dram_tensor("ones_d", [P, P], bf16, kind="ExternalInput")[:, :]
    wq = nc.dram_tensor("wq", [D, E], bf16, kind="ExternalInput")[:, :]
    wk = nc.dram_tensor("wk", [D, E], bf16, kind="ExternalInput")[:, :]
    wv = nc.dram_tensor("wv", [D, E], bf16, kind="ExternalInput")[:, :]
    ow = nc.dram_tensor("ow", [E, D], bf16, kind="ExternalInput")[:, :]
    bq_col = nc.dram_tensor("bq_col", [P, ET], f32, kind="ExternalInput")[:, :]
    bk_col = nc.dram_tensor("bk_col", [P, ET], f32, kind="ExternalInput")[:, :]
    bv_bc = nc.dram_tensor("bv_bc", [P, E], bf16, kind="ExternalInput")[:, :]
    ob_bc = nc.dram_tensor("ob_bc", [P, D], f32, kind="ExternalInput")[:, :]
    # delta-row selector + ob with row 0 = out bias: the final out group
    # folds its bias in via matmul so the evac is a plain ACT copy
    ob_sel = nc.dram_tensor("ob_sel", [P, P], bf16, kind="ExternalInput")[:, :]
    ob_mat = nc.dram_tensor("ob_mat", [P, D], bf16, kind="ExternalInput")[:, :]
    out = nc.dram_tensor("out", [QSH, D], f32, kind="ExternalOutput")[:, :]

    qT_r = qT.rearrange("(t p) n -> p t n", p=P)   # [128, DT, QSH]
    kT_r = kT.rearrange("(t p) n -> p t n", p=P)
    vT_r = vT.rearrange("(t p) n -> p t n", p=P)
    wq_r = wq.rearrange("(t p) n -> p t n", p=P)   # [128, DT, E]
    wk_r = wk.rearrange("(t p) n -> p t n", p=P)
    wv_r = wv.rearrange("(t p) n -> p t n", p=P)
    ow_r = ow.rearrange("(t p) n -> p t n", p=P)   # [128, ET, D]

    def mm(ps, lhsT, rhs, start, stop):
        nc.tensor.matmul(ps, lhsT, rhs, start=start, stop=stop)

    NWARM = 10

    with tile.TileContext(nc) as tc:
        # ---- persistent smalls (tiles allocated here; DMAs emitted inside
        # the AB scope so the scalar queue prioritizes wv chunks) ----
        with tc.tile_pool(name="smalls", bufs=1) as smalls:
            bv_t = smalls.tile([P, E], bf16, name="bv_t")
            mask_t = smalls.tile([P, KT], f32, name="maskc")
            bk_t = smalls.tile([P, ET], f32, name="bkc")
            bq_t = smalls.tile([P, ET], f32, name="bqc")
            ones_t = smalls.tile([P, P], bf16, name="ones")

            # persistent operand tensors
            with tc.tile_pool(name="wqp", bufs=1) as wqp, \
                 tc.tile_pool(name="vpp", bufs=1) as vpp, \
                 tc.tile_pool(name="kpp", bufs=1) as kpp, \
                 tc.tile_pool(name="qpp", bufs=1) as qpp, \
                 tc.tile_pool(name="expp", bufs=1) as expp, \
                 tc.tile_pool(name="ctxp", bufs=1) as ctxp:
                wq_t = wqp.tile([P, DT, E], bf16, name="wq_t")
                vp = vpp.tile([P, KT, E], bf16, name="vp")      # [k, E]
                kp = kpp.tile([P, ET, SK], bf16, name="kp")     # [E, k] (kp^T)
                qps = [qpp.tile([P, ET, QB], bf16, name=f"qp{i}")
                       for i in range(NQB)]                      # [E, q] (qp^T)
                expT = expp.tile([P, KT, QB], bf16, name="expT")  # [k, q]
                ctxs = [ctxp.tile([P, ET, QB], bf16, name=f"ctx{i}")
                        for i in range(NQB)]                     # [E, q] (ctx^T)

                # ============ phase A+B: vp then kp (sibling pools so kp
                # weights prefetch during vp) ============
                with tc.tile_pool(name="wv_w", bufs=1) as wvp, \
                     tc.tile_pool(name="wk_w", bufs=1) as wkp, \
                     tc.tile_pool(name="vT_s", bufs=2) as vts, \
                     tc.tile_pool(name="kT_s", bufs=2) as kts, \
                     tc.tile_pool(name="qT0_s", bufs=1) as qt0s, \
                     tc.tile_pool(name="warm", bufs=1) as warm, \
                     tc.tile_pool(name="warm_ps", bufs=1,
                                  space="PSUM") as wps, \
                     tc.tile_pool(name="ab_ps", bufs=3, space="PSUM") as abps:
                    # PE warm-up: dummy matmuls on a memset tile fill the
                    # ~5.7us wait for the first weight DMA and ramp the PE
                    # p-state so real work starts at full clock. Lives in
                    # the AB scope so nothing aliases (and WAR-waits on) it.
                    junk = warm.tile([P, 512], bf16, name="junk")
                    nc.vector.memset(junk[:], 0.0)
                    wp = wps.tile([P, 512], f32, name="warmps")
                    for i in range(NWARM):
                        nc.tensor.matmul(wp[:], junk[:, 0:P], junk[:],
                                         start=(i == 0),
                                         stop=(i == NWARM - 1))
                    # The shared DMA bus serves transfers in descriptor-gen
                    # completion order, so every queue is sequenced by first
                    # NEED: sync gets only the first wv half; Pool (slow 1.3us
                    # SWDGE gen each = natural pacing) carries the whole
                    # vp/kp-phase stream in consumption order; scalar gets the
                    # smalls then the late-needed wq/qt0.
                    wv_t = wvp.tile([P, DT, E], bf16, name="wv_t")
                    wk_t = wkp.tile([P, DT, E], bf16, name="wk_t")
                    nc.sync.dma_start(wv_t[:, 0:4, 0:512], wv_r[:, 0:4, 0:512])
                    nc.scalar.dma_start(wv_t[:, 4:8, 0:512],
                                        wv_r[:, 4:8, 0:512])
                    # smalls on scalar, ordered by first use (both bv halves
                    # ride the Pool queue so they can't delay the vT stream)
                    nc.scalar.dma_start(mask_t[:], mask_cols)
                    nc.scalar.mul(mask_t[:], mask_t[:], NEG)
                    nc.scalar.dma_start(bk_t[:], bk_col)
                    nc.scalar.dma_start(bq_t[:], bq_col)
                    nc.scalar.mul(bq_t[:], bq_t[:], ISCALE)
                    nc.scalar.dma_start(ones_t[:], ones_d)
                    qt0 = qt0s.tile([P, DT, QB], bf16, name="qt0")

                    # -- vp: psum [128k, 512E] per (m, n) group; vT streamed
                    # in 1MB chunks of 4 k-tiles (desc count is per (p,t),
                    # so wider chunks halve Pool SWDGE time) --
                    vtiles = {}

                    def load_vt(c):
                        vt = vts.tile([P, DT, 4 * P], bf16, tag="vt",
                                      name=f"vt{c}")
                        if c == 0:
                            # split so each early k-tile unblocks as it lands
                            nc.gpsimd.dma_start(vt[:, :, 0:P],
                                                vT_r[:, :, 0:P])
                            nc.gpsimd.dma_start(vt[:, :, P:2 * P],
                                                vT_r[:, :, P:2 * P])
                            nc.gpsimd.dma_start(vt[:, :, 2 * P:4 * P],
                                                vT_r[:, :, 2 * P:4 * P])
                        else:
                            nc.gpsimd.dma_start(
                                vt[:], vT_r[:, :, 4 * c * P:(4 * c + 4) * P])
                        vtiles[c] = vt

                    def vp_group(m, n):
                        ps = abps.tile([P, 512], f32, tag="ps",
                                       name=f"vps{m}_{n}")
                        vt = vtiles[m // 4]
                        mi = m % 4
                        for t in range(DT):
                            mm(ps[:], vt[:, t, mi * P:(mi + 1) * P],
                               wv_t[:, t, n * 512:(n + 1) * 512],
                               t == 0, t == DT - 1)
                        nc.vector.tensor_add(
                            vp[:, m, n * 512:(n + 1) * 512], ps[:],
                            bv_t[:, n * 512:(n + 1) * 512])

                    # chunk 0: n=0 groups first (wv col half 1 still loading)
                    load_vt(0)
                    # Pool/SWDGE queue carries everything else in strict
                    # consumption order (scalar/sync are HWDGE-fast and would
                    # let late-needed weights jump the shared bus)
                    nc.gpsimd.dma_start(bv_t[:, 0:512], bv_bc[:, 0:512])
                    nc.gpsimd.dma_start(wv_t[:, :, 512:1024],
                                        wv_r[:, :, 512:1024])
                    nc.gpsimd.dma_start(bv_t[:, 512:1024], bv_bc[:, 512:1024])
                    load_vt(1)
                    for m in range(4):
                        vp_group(m, 0)
                    for m in range(4):
                        vp_group(m, 1)
                    for c in range(1, KT // 4):
                        if c + 1 < KT // 4:
                            load_vt(c + 1)
                        for mi in range(4):
                            for n in range(E // 512):
                                vp_group(4 * c + mi, n)
                    # wk lands on the bus after vt2/vt3 (needed at kp start)
                    nc.gpsimd.dma_start(wk_t[:, :, 0:512], wk_r[:, :, 0:512])
                    nc.gpsimd.dma_start(wk_t[:, :, 512:1024],
                                        wk_r[:, :, 512:1024])

                    # -- kp: for each k-chunk, psum [128E, 512k] x8 --
                    for n in range(SK // KNB):
                        kt = kts.tile([P, DT, KNB], bf16, tag="kt",
                                      name=f"kt{n}")
                        nc.gpsimd.dma_start(kt[:],
                                            kT_r[:, :, n * KNB:(n + 1) * KNB])
                        for m in range(ET):
                            ps = abps.tile([P, KNB], f32, tag="ps",
                                           name=f"kps{n}_{m}")
                            for t in range(DT):
                                mm(ps[:], wk_t[:, t, m * P:(m + 1) * P],
                                   kt[:, t, :], t == 0, t == DT - 1)
                            nc.scalar.activation(
                                kp[:, m, n * KNB:(n + 1) * KNB], ps[:],
                                AF.Identity, bias=bk_t[:, m:m + 1])

                    # wq/qt0 queue behind the kp stream; kt2/kt3's WAR head-
                    # of-line block paces their generation to ~90us, arriving
                    # in time for qp0 at ~116us
                    nc.gpsimd.dma_start(wq_t[:, 0:4, :], wq_r[:, 0:4, :])
                    nc.gpsimd.dma_start(wq_t[:, 4:8, :], wq_r[:, 4:8, :])
                    nc.gpsimd.dma_start(qt0[:], qT_r[:, :, 0:QB])

                    # -- qp(qb0): psum [128E, 512q] x8 (reuse ab psum bufs) --
                    for m in range(ET):
                        ps = abps.tile([P, QB], f32, tag="ps", name=f"qps0_{m}")
                        for t in range(DT):
                            mm(ps[:], wq_t[:, t, m * P:(m + 1) * P],
                               qt0[:, t, :], t == 0, t == DT - 1)
                        nc.scalar.activation(qps[0][:, m, :], ps[:],
                                             AF.Identity,
                                             bias=bq_t[:, m:m + 1],
                                             scale=ISCALE)

                # ============ attention (ow/qT1/out-staging reuse AB space) ==
                with tc.tile_pool(name="ow_w", bufs=1) as owp, \
                     tc.tile_pool(name="qT1_s", bufs=1) as qt1s, \
                     tc.tile_pool(name="obp", bufs=1) as obp, \
                     tc.tile_pool(name="smx", bufs=1) as smx, \
                     tc.tile_pool(name="outsb", bufs=4) as osb:
                    # single recip/acc/accb tiles shared across q-blocks
                    # (their lifetimes don't overlap). acc is the DVE-side
                    # softmax-sum accumulator (f32); accb its bf16 staging
                    # for the final cross-partition ones-matmul: moves the
                    # per-kb sum matmul (16384 cycles) off the PE
                    recip_t = smx.tile([P, QB], f32, name="recip")
                    acc_t = smx.tile([P, QB], f32, name="acc")
                    accb_t = smx.tile([P, QB], bf16, name="accb")
                    # Pool/SWDGE queue again: these generate after the kp
                    # stream + wq/qt0, landing well before first use (~160us+)
                    qt1 = qt1s.tile([P, DT, QB], bf16, name="qt1")
                    nc.gpsimd.dma_start(qt1[:], qT_r[:, :, QB:2 * QB])
                    ow_t = owp.tile([P, ET, D], bf16, name="ow_t")
                    nc.gpsimd.dma_start(ow_t[:, 0:4, :], ow_r[:, 0:4, :])
                    nc.gpsimd.dma_start(ow_t[:, 4:8, :], ow_r[:, 4:8, :])
                    ob_t = obp.tile([P, D], f32, name="ob_t")
                    nc.gpsimd.dma_start(ob_t[:], ob_bc)
                    obsel_t = obp.tile([P, P], bf16, name="obsel_t")
                    nc.gpsimd.dma_start(obsel_t[:], ob_sel)
                    obmat_t = obp.tile([P, D], bf16, name="obmat_t")
                    nc.gpsimd.dma_start(obmat_t[:], ob_mat)

                    def out_group(qb, g, spare_pool, store_eng=None):
                        """out[qb*QB+mq*128 : +128, nd*512 : +512] (8 mm)."""
                        nd, mq = divmod(g, MQ)
                        ps = spare_pool.tile([P, DNB], f32, tag="sp",
                                             name=f"ops{qb}_{g}")
                        for e in range(ET):
                            mm(ps[:], ctxs[qb][:, e, mq * P:(mq + 1) * P],
                               ow_t[:, e, nd * DNB:(nd + 1) * DNB],
                               e == 0, e == ET - 1)
                        r0 = qb * QB + mq * P
                        ot = osb.tile([P, DNB], f32, tag="ot",
                                      name=f"ot{qb}_{g}")
                        nc.vector.tensor_add(
                            ot[:], ps[:], ob_t[:, nd * DNB:(nd + 1) * DNB])
                        (store_eng or nc.gpsimd).dma_start(
                            out[r0:r0 + P, nd * DNB:(nd + 1) * DNB], ot[:])

                    def out_group_tail(qb, g, tail_pool):
                        """Last group: bias folded in via the delta-row
                        matmul, evac by plain ACT copy (no DVE bias add on
                        the critical path), 2 column chunks so chunk 0's
                        store overlaps chunk 1's matmuls; final store rides
                        the fast HWDGE queue."""
                        nd, mq = divmod(g, MQ)
                        r0 = qb * QB + mq * P
                        engs = [nc.sync, nc.scalar]
                        widths = [3 * DNB // 4, DNB // 4]
                        for j in range(2):
                            c0 = nd * DNB + j * widths[0]
                            w = widths[j]
                            ps = tail_pool.tile([P, w], f32, tag=f"tp{j}",
                                                name=f"opst{qb}_{g}_{j}")
                            ot = osb.tile([P, w], f32, tag=f"ott{j}",
                                          name=f"ott{qb}_{g}_{j}")
                            if j == 0:
                                # DVE bias-add evac so the two chunks'
                                # evacuations run on different engines
                                for e in range(ET):
                                    mm(ps[:],
                                       ctxs[qb][:, e, mq * P:(mq + 1) * P],
                                       ow_t[:, e, c0:c0 + w],
                                       e == 0, e == ET - 1)
                                nc.vector.tensor_add(
                                    ot[:], ps[:], ob_t[:, c0:c0 + w])
                            else:
                                mm(ps[:], obsel_t[:], obmat_t[:, c0:c0 + w],
                                   True, False)
                                for e in range(ET):
                                    mm(ps[:],
                                       ctxs[qb][:, e, mq * P:(mq + 1) * P],
                                       ow_t[:, e, c0:c0 + w],
                                       False, e == ET - 1)
                                nc.scalar.activation(ot[:], ps[:], AF.Identity)
                            engs[j].dma_start(
                                out[r0:r0 + P, c0:c0 + w], ot[:])

                    for qb in range(NQB):
                        # banks 0-3: ctx half1 accumulators (live whole block)
                        cps_cm = tc.tile_pool(name=f"cps{qb}", bufs=1,
                                              space="PSUM")
                        cpsp = cps_cm.__enter__()
                        cps = [cpsp.tile([P, QB], f32, name=f"c{qb}_{e}")
                               for e in range(ET // 2)]
                        # banks 4-7: spare + logits triple-buffer. Open order
                        # matters: the allocator hands the most-recently-freed
                        # banks to the first-opened pool, and the previous
                        # qb's last-freed banks (ctx-half2, evacuated latest)
                        # must NOT land on lg, whose first use is immediate.
                        # spare's first use (kb>=1) absorbs that latency.
                        with tc.tile_pool(name=f"spare{qb}", bufs=1,
                                          space="PSUM") as spp, \
                             tc.tile_pool(name=f"lg{qb}", bufs=3,
                                          space="PSUM") as lgp:

                            def lg_mm(kb):
                                ps = lgp.tile([P, QB], f32, tag="lg",
                                              name=f"lg{qb}_{kb}")
                                for e in range(ET):
                                    mm(ps[:], kp[:, e, kb * P:(kb + 1) * P],
                                       qps[qb][:, e, :], e == 0, e == ET - 1)
                                nc.scalar.activation(
                                    expT[:, kb, :], ps[:], AF.Exp,
                                    bias=mask_t[:, kb:kb + 1])
                                # fold exp tile into the DVE softmax-sum acc
                                if kb == 0:
                                    nc.vector.tensor_copy(acc_t[:],
                                                          expT[:, 0, :])
                                else:
                                    nc.vector.tensor_add(acc_t[:],
                                                         acc_t[:],
                                                         expT[:, kb, :])

                            def tail_mm(kb):
                                for e in range(ET // 2):
                                    mm(cps[e][:],
                                       vp[:, kb, e * P:(e + 1) * P],
                                       expT[:, kb, :], kb == 0, kb == KT - 1)

                            for kb in range(KT):
                                lg_mm(kb)
                                if kb > 0:
                                    tail_mm(kb - 1)
                                # spare-bank work, one group per slot:
                                if qb == 0 and kb >= ET:
                                    # qp(qb1) group m = kb-8
                                    m = kb - ET
                                    ps = spp.tile([P, QB], f32, tag="sp",
                                                  name=f"qps1_{m}")
                                    for t in range(DT):
                                        mm(ps[:], wq_t[:, t, m * P:(m + 1) * P],
                                           qt1[:, t, :], t == 0, t == DT - 1)
                                    nc.scalar.activation(
                                        qps[1][:, m, :], ps[:], AF.Identity,
                                        bias=bq_t[:, m:m + 1], scale=ISCALE)
                                if qb == 1 and kb % 2 == 1:
                                    out_group(0, kb // 2, spp)
                            tail_mm(KT - 1)

                        # ctx half2 on a 2-bank ping-pong: only two banks
                        # inherit late evacuations, so the next qb's logits
                        # pool (first-fit) lands on early-freed banks and
                        # starts without waiting. One more bank carries the
                        # cross-partition ones-matmul of the DVE sum acc.
                        with tc.tile_pool(name=f"sum{qb}", bufs=1,
                                          space="PSUM") as sump, \
                             tc.tile_pool(name=f"cps2_{qb}", bufs=2,
                                          space="PSUM") as cps2p:
                            nc.vector.tensor_copy(accb_t[:], acc_t[:])
                            s_ps = sump.tile([P, QB], f32, name=f"sps{qb}")
                            mm(s_ps[:], ones_t[:], accb_t[:],
                               True, True)
                            nc.vector.reciprocal(recip_t[:], s_ps[:])
                            # evacuate half1 (DVE) while half2 accumulates
                            for e in range(ET // 2):
                                nc.vector.tensor_mul(ctxs[qb][:, e, :],
                                                     cps[e][:],
                                                     recip_t[:])
                            for ei in range(ET // 2):
                                e = ET // 2 + ei
                                c2 = cps2p.tile([P, QB], f32, tag="c2",
                                                name=f"c2_{qb}_{e}")
                                for kb in range(KT):
                                    mm(c2[:],
                                       vp[:, kb, e * P:(e + 1) * P],
                                       expT[:, kb, :], kb == 0, kb == KT - 1)
                                # evac right away so the bank frees for the
                                # next sweep / next qb's pools
                                nc.vector.tensor_mul(ctxs[qb][:, e, :],
                                                     c2[:],
                                                     recip_t[:])
                        cps_cm.__exit__(None, None, None)

                    # ---- final out phase: out(qb1) ----
                    with tc.tile_pool(name="out_ps", bufs=2,
                                      space="PSUM") as ops, \
                         tc.tile_pool(name="tail_ps", bufs=1,
                                      space="PSUM") as tps:
                        # final-phase stores ride the idle HWDGE queues
                        # (scalar/sync) so Pool has nothing to drain at the
                        # kernel tail
                        for g in range(ND * MQ - 1):
                            out_group(1, g, ops,
                                      store_eng=(nc.sync, nc.scalar)[g % 2])
                        out_group_tail(1, ND * MQ - 1, tps)

    nc.compile()
    return nc


def make_in_maps(v, k, q, mask, wq_w, wq_b, wk_w, wk_b, wv_w, wv_b, out_w, out_b,
                 n_cores=8, D=1024, E=1024, SK=2048, QSH=1024):
    """Host-side shard + layout prep (data movement + bf16 cast, no math)."""
    import ml_dtypes
    bf = ml_dtypes.bfloat16
    ET = E // P
    KT = SK // P
    f = np.float32
    wq_w = np.ascontiguousarray(np.asarray(wq_w, f).astype(bf))
    wk_w = np.ascontiguousarray(np.asarray(wk_w, f).astype(bf))
    wv_w = np.ascontiguousarray(np.asarray(wv_w, f).astype(bf))
    out_w = np.ascontiguousarray(np.asarray(out_w, f).astype(bf))
    bq_col = np.ascontiguousarray(np.asarray(wq_b, f).reshape(ET, P).T)
    bk_col = np.ascontiguousarray(np.asarray(wk_b, f).reshape(ET, P).T)
    bv_bc = np.ascontiguousarray(
        np.broadcast_to(np.asarray(wv_b, f).astype(bf), (P, E)))
    ob_bc = np.ascontiguousarray(
        np.broadcast_to(np.asarray(out_b, f), (P, len(out_b))))
    ones_arr = np.ones((P, P), bf)
    ob_sel = np.zeros((P, P), bf)
    ob_sel[0, :] = 1
    ob_mat = np.zeros((P, len(out_b)), bf)
    ob_mat[0, :] = np.asarray(out_b, f).astype(bf)
    in_maps = []
    for c in range(n_cores):
        b, h = divmod(c, 2)
        qTc = np.ascontiguousarray(
            np.asarray(q[b, h * QSH:(h + 1) * QSH, :], f).T.astype(bf))
        kTc = np.ascontiguousarray(np.asarray(k[b], f).T.astype(bf))
        vTc = np.ascontiguousarray(np.asarray(v[b], f).T.astype(bf))
        mc = np.ascontiguousarray(np.asarray(mask[b, 0], f).reshape(KT, P).T)
        in_maps.append(dict(qT=qTc, kT=kTc, vT=vTc, mask_cols=mc,
                            ones_d=ones_arr,
                            wq=wq_w, wk=wk_w, wv=wv_w, ow=out_w,
                            bq_col=bq_col, bk_col=bk_col,
                            bv_bc=bv_bc, ob_bc=ob_bc,
                            ob_sel=ob_sel, ob_mat=ob_mat))
    return in_maps


_NC_CACHE = {}


def kernel(v, k, q, mask, wq_w, wq_b, wk_w, wk_b, wv_w, wv_b, out_w, out_b):
    from concourse.bass_utils import run_bass_kernel_spmd

    B, S, D = 4, 2048, 1024
    E, QSH = 1024, 1024
    if "nc" not in _NC_CACHE:
        _NC_CACHE["nc"] = build_nc(D=D, E=E, SK=S, QSH=QSH, QB=512)
    nc = _NC_CACHE["nc"]

    in_maps = make_in_maps(v, k, q, mask, wq_w, wq_b, wk_w, wk_b, wv_w, wv_b,
                           out_w, out_b, n_cores=8, D=D, E=E, SK=S, QSH=QSH)
    trace = bool(int(os.environ.get("BASS_KERNEL_TRACE", "0")))
    res = run_bass_kernel_spmd(nc, in_maps, core_ids=list(range(8)), trace=trace)
    if trace:
        print(f"HW exec time: {res.exec_time_ns} ns")
        _NC_CACHE["last_exec_time_ns"] = res.exec_time_ns
        _NC_CACHE["last_trace"] = res.instructions_and_trace

    outp = np.empty((B, S, D), np.float32)
    for c in range(8):
        b, h = divmod(c, 2)
        outp[b, h * QSH:(h + 1) * QSH, :] = res.results[c]["out"]
    return outp



# revision 29
# speedup vs baseline: 1.0034x; 1.0028x over previous
"""Single-head attention (B=4, S=2048, D=E=1024) on 8 trn2 NeuronCores.

Sharding: data-parallel over (batch, q-half) -> 8 shards. Each core gets a
1024-row q shard plus the full 2048 keys of its batch; K/V projections are
recomputed on both cores of a batch pair (25% extra flops, zero collectives).

All matmul operands are bf16 (host-cast); PSUM accumulation stays fp32, so
per-value RMS error ~0.1% -- far inside the 2e-2 gate. bf16 runs at the same
1 cycle/row PE rate as fp32r but halves DMA + SBUF, which lets every weight
stay resident (no DRAM bounce) and keeps the PE streaming continuously:

  per-core PE work (cycles @2.4GHz):
    vp 131072 + kp 131072 + qp 65536 + logits 131072 + ctx 131072
    + out 65536 + sum-reduce 1024 = 656k cycles = 273.5us ideal
  (the per-kb softmax-sum ones-matmuls run on DVE as f32 adds instead,
   leaving PE only one 128-partition ones-reduce per q-block)

Schedule: vp -> kp -> qp(qb0) -> qb0 kb-loop [logits|exp|ctx-half1 on PE,
sum-acc on DVE, qp(qb1) in the spare PSUM bank] -> ctx-half2 -> qb1 kb-loop
[with out(qb0) in the spare bank] -> ctx-half2 -> out(qb1, stores on the
idle HWDGE queues; tail chunks evacuate on DVE||ACT in parallel). PSUM never
exceeds 8 banks; weights for each phase are prefetched during the previous
phase via sibling pools.
"""

import os
import numpy as np

P = 128
NEG = -1.0e9


def build_nc(D=1024, E=1024, SK=2048, QSH=1024, QB=512):
    """Build the per-core Bass module (SPMD; same program on all cores)."""
    import concourse.bass as bass
    import concourse.mybir as mybir
    import concourse.tile as tile
    from concourse import bacc

    f32 = mybir.dt.float32
    bf16 = mybir.dt.bfloat16
    AF = mybir.ActivationFunctionType

    DT = D // P          # contraction tiles over model dim
    ET = E // P          # enc tiles
    KT = SK // P         # key tiles
    NQB = QSH // QB      # q blocks (2)
    KNB = 512            # key free-dim block for kp
    DNB = 512            # model free-dim block for out
    MQ = QB // P         # q 128-row groups per block (4)
    ND = D // DNB        # out column chunks (2)
    ISCALE = 1.0 / float(np.sqrt(E))

    nc = bacc.Bacc(trn_type="TRN2")

    # ---- I/O (bf16 operands; f32 biases/mask; f32 output) ----
    qT = nc.dram_tensor("qT", [D, QSH], bf16, kind="ExternalInput")[:, :]
    kT = nc.dram_tensor("kT", [D, SK], bf16, kind="ExternalInput")[:, :]
    vT = nc.dram_tensor("vT", [D, SK], bf16, kind="ExternalInput")[:, :]
    mask_cols = nc.dram_tensor("mask_cols", [P, KT], f32, kind="ExternalInput")[:, :]
    ones_d = nc.dram_tensor("ones_d", [P, P], bf16, kind="ExternalInput")[:, :]
    wq = nc.dram_tensor("wq", [D, E], bf16, kind="ExternalInput")[:, :]
    wk = nc.dram_tensor("wk", [D, E], bf16, kind="ExternalInput")[:, :]
    wv = nc.dram_tensor("wv", [D, E], bf16, kind="ExternalInput")[:, :]
    ow = nc.dram_tensor("ow", [E, D], bf16, kind="ExternalInput")[:, :]
    bq_col = nc.dram_tensor("bq_col", [P, ET], f32, kind="ExternalInput")[:, :]
    bk_col = nc.dram_tensor("bk_col", [P, ET], f32, kind="ExternalInput")[:, :]
    bv_bc = nc.dram_tensor("bv_bc", [P, E], bf16, kind="ExternalInput")[:, :]
    ob_bc = nc.dram_tensor("ob_bc", [P, D], f32, kind="ExternalInput")[:, :]
    # delta-row selector + ob with row 0 = out bias: the final out group
    # folds its bias in via matmul so the evac is a plain ACT copy
    ob_sel = nc.dram_tensor("ob_sel", [P, P], bf16, kind="ExternalInput")[:, :]
    ob_mat = nc.dram_tensor("ob_mat", [P, D], bf16, kind="ExternalInput")[:, :]
    out = nc.dram_tensor("out", [QSH, D], bf16, kind="ExternalOutput")[:, :]

    qT_r = qT.rearrange("(t p) n -> p t n", p=P)   # [128, DT, QSH]
    kT_r = kT.rearrange("(t p) n -> p t n", p=P)
    vT_r = vT.rearrange("(t p) n -> p t n", p=P)
    wq_r = wq.rearrange("(t p) n -> p t n", p=P)   # [128, DT, E]
    wk_r = wk.rearrange("(t p) n -> p t n", p=P)
    wv_r = wv.rearrange("(t p) n -> p t n", p=P)
    ow_r = ow.rearrange("(t p) n -> p t n", p=P)   # [128, ET, D]

    def mm(ps, lhsT, rhs, start, stop):
        nc.tensor.matmul(ps, lhsT, rhs, start=start, stop=stop)

    NWARM = 10

    with tile.TileContext(nc) as tc:
        # ---- persistent smalls (tiles allocated here; DMAs emitted inside
        # the AB scope so the scalar queue prioritizes wv chunks) ----
        with tc.tile_pool(name="smalls", bufs=1) as smalls:
            bv_t = smalls.tile([P, E], bf16, name="bv_t")
            mask_t = smalls.tile([P, KT], f32, name="maskc")
            bk_t = smalls.tile([P, ET], f32, name="bkc")
            bq_t = smalls.tile([P, ET], f32, name="bqc")
            ones_t = smalls.tile([P, P], bf16, name="ones")

            # persistent operand tensors
            with tc.tile_pool(name="wqp", bufs=1) as wqp, \
                 tc.tile_pool(name="vpp", bufs=1) as vpp, \
                 tc.tile_pool(name="kpp", bufs=1) as kpp, \
                 tc.tile_pool(name="qpp", bufs=1) as qpp, \
                 tc.tile_pool(name="expp", bufs=1) as expp, \
                 tc.tile_pool(name="ctxp", bufs=1) as ctxp:
                wq_t = wqp.tile([P, DT, E], bf16, name="wq_t")
                vp = vpp.tile([P, KT, E], bf16, name="vp")      # [k, E]
                kp = kpp.tile([P, ET, SK], bf16, name="kp")     # [E, k] (kp^T)
                qps = [qpp.tile([P, ET, QB], bf16, name=f"qp{i}")
                       for i in range(NQB)]                      # [E, q] (qp^T)
                expT = expp.tile([P, KT, QB], bf16, name="expT")  # [k, q]
                ctxs = [ctxp.tile([P, ET, QB], bf16, name=f"ctx{i}")
                        for i in range(NQB)]                     # [E, q] (ctx^T)

                # ============ phase A+B: vp then kp (sibling pools so kp
                # weights prefetch during vp) ============
                with tc.tile_pool(name="wv_w", bufs=1) as wvp, \
                     tc.tile_pool(name="wk_w", bufs=1) as wkp, \
                     tc.tile_pool(name="vT_s", bufs=2) as vts, \
                     tc.tile_pool(name="kT_s", bufs=2) as kts, \
                     tc.tile_pool(name="qT0_s", bufs=1) as qt0s, \
                     tc.tile_pool(name="warm", bufs=1) as warm, \
                     tc.tile_pool(name="warm_ps", bufs=1,
                                  space="PSUM") as wps, \
                     tc.tile_pool(name="ab_ps", bufs=3, space="PSUM") as abps:
                    # PE warm-up: dummy matmuls on a memset tile fill the
                    # ~5.7us wait for the first weight DMA and ramp the PE
                    # p-state so real work starts at full clock. Lives in
                    # the AB scope so nothing aliases (and WAR-waits on) it.
                    junk = warm.tile([P, 512], bf16, name="junk")
                    nc.vector.memset(junk[:], 0.0)
                    wp = wps.tile([P, 512], f32, name="warmps")
                    for i in range(NWARM):
                        nc.tensor.matmul(wp[:], junk[:, 0:P], junk[:],
                                         start=(i == 0),
                                         stop=(i == NWARM - 1))
                    # The shared DMA bus serves transfers in descriptor-gen
                    # completion order, so every queue is sequenced by first
                    # NEED: sync gets only the first wv half; Pool (slow 1.3us
                    # SWDGE gen each = natural pacing) carries the whole
                    # vp/kp-phase stream in consumption order; scalar gets the
                    # smalls then the late-needed wq/qt0.
                    wv_t = wvp.tile([P, DT, E], bf16, name="wv_t")
                    wk_t = wkp.tile([P, DT, E], bf16, name="wk_t")
                    nc.sync.dma_start(wv_t[:, 0:4, 0:512], wv_r[:, 0:4, 0:512])
                    nc.scalar.dma_start(wv_t[:, 4:8, 0:512],
                                        wv_r[:, 4:8, 0:512])
                    qt0 = qt0s.tile([P, DT, QB], bf16, name="qt0")

                    # -- vp: psum [128k, 512E] per (m, n) group; vT streamed
                    # in 1MB chunks of 4 k-tiles (desc count is per (p,t),
                    # so wider chunks halve Pool SWDGE time) --
                    vtiles = {}

                    def load_vt(c):
                        vt = vts.tile([P, DT, 4 * P], bf16, tag="vt",
                                      name=f"vt{c}")
                        if c == 0:
                            # split so early k-tiles unblock as they land.
                            # 256-col pieces keep the per-descriptor run at
                            # 512B -- a 128-col piece has 256B runs and pays
                            # the DMA model's 2x sub-512B latency multiplier
                            nc.gpsimd.dma_start(vt[:, :, 0:2 * P],
                                                vT_r[:, :, 0:2 * P])
                            nc.gpsimd.dma_start(vt[:, :, 2 * P:4 * P],
                                                vT_r[:, :, 2 * P:4 * P])
                        else:
                            nc.gpsimd.dma_start(
                                vt[:], vT_r[:, :, 4 * c * P:(4 * c + 4) * P])
                        vtiles[c] = vt

                    def vp_group(m, n):
                        ps = abps.tile([P, 512], f32, tag="ps",
                                       name=f"vps{m}_{n}")
                        vt = vtiles[m // 4]
                        mi = m % 4
                        for t in range(DT):
                            mm(ps[:], vt[:, t, mi * P:(mi + 1) * P],
                               wv_t[:, t, n * 512:(n + 1) * 512],
                               t == 0, t == DT - 1)
                        nc.vector.tensor_add(
                            vp[:, m, n * 512:(n + 1) * 512], ps[:],
                            bv_t[:, n * 512:(n + 1) * 512])

                    # chunk 0: n=0 groups first (wv col half 1 still loading)
                    load_vt(0)
                    # Pool/SWDGE queue carries everything else in strict
                    # consumption order (scalar/sync are HWDGE-fast and would
                    # let late-needed weights jump the shared bus)
                    nc.gpsimd.dma_start(bv_t[:, 0:512], bv_bc[:, 0:512])
                    nc.gpsimd.dma_start(wv_t[:, :, 512:1024],
                                        wv_r[:, :, 512:1024])
                    nc.gpsimd.dma_start(bv_t[:, 512:1024], bv_bc[:, 512:1024])
                    load_vt(1)
                    for m in range(4):
                        vp_group(m, 0)
                    for m in range(4):
                        vp_group(m, 1)
                    for c in range(1, KT // 4):
                        if c + 1 < KT // 4:
                            load_vt(c + 1)
                        for mi in range(4):
                            for n in range(E // 512):
                                vp_group(4 * c + mi, n)
                    # smalls on scalar, emitted after the vp stream so their
                    # descs land on the bus behind the critical wv/vt0 set
                    # (first use: bk ~35us, bq ~75us, mask ~90us, ones ~180us)
                    nc.scalar.dma_start(bk_t[:], bk_col)
                    nc.scalar.dma_start(mask_t[:], mask_cols)
                    nc.scalar.mul(mask_t[:], mask_t[:], NEG)
                    nc.scalar.dma_start(bq_t[:], bq_col)
                    nc.scalar.mul(bq_t[:], bq_t[:], ISCALE)
                    nc.scalar.dma_start(ones_t[:], ones_d)
                    # wk lands on the bus after vt2/vt3 (needed at kp start)
                    nc.gpsimd.dma_start(wk_t[:, :, 0:512], wk_r[:, :, 0:512])
                    nc.gpsimd.dma_start(wk_t[:, :, 512:1024],
                                        wk_r[:, :, 512:1024])

                    # -- kp: for each k-chunk, psum [128E, 512k] x8 --
                    for n in range(SK // KNB):
                        kt = kts.tile([P, DT, KNB], bf16, tag="kt",
                                      name=f"kt{n}")
                        nc.gpsimd.dma_start(kt[:],
                                            kT_r[:, :, n * KNB:(n + 1) * KNB])
                        for m in range(ET):
                            ps = abps.tile([P, KNB], f32, tag="ps",
                                           name=f"kps{n}_{m}")
                            for t in range(DT):
                                mm(ps[:], wk_t[:, t, m * P:(m + 1) * P],
                                   kt[:, t, :], t == 0, t == DT - 1)
                            nc.scalar.activation(
                                kp[:, m, n * KNB:(n + 1) * KNB], ps[:],
                                AF.Identity, bias=bk_t[:, m:m + 1])

                    # wq/qt0 queue behind the kp stream; kt2/kt3's WAR head-
                    # of-line block paces their generation to ~90us, arriving
                    # in time for qp0 at ~116us
                    nc.gpsimd.dma_start(wq_t[:, 0:4, :], wq_r[:, 0:4, :])
                    nc.gpsimd.dma_start(wq_t[:, 4:8, :], wq_r[:, 4:8, :])
                    nc.gpsimd.dma_start(qt0[:], qT_r[:, :, 0:QB])

                    # -- qp(qb0): psum [128E, 512q] x8 (reuse ab psum bufs) --
                    for m in range(ET):
                        ps = abps.tile([P, QB], f32, tag="ps", name=f"qps0_{m}")
                        for t in range(DT):
                            mm(ps[:], wq_t[:, t, m * P:(m + 1) * P],
                               qt0[:, t, :], t == 0, t == DT - 1)
                        nc.scalar.activation(qps[0][:, m, :], ps[:],
                                             AF.Identity,
                                             bias=bq_t[:, m:m + 1],
                                             scale=ISCALE)

                # ============ attention (ow/qT1/out-staging reuse AB space) ==
                with tc.tile_pool(name="ow_w", bufs=1) as owp, \
                     tc.tile_pool(name="qT1_s", bufs=1) as qt1s, \
                     tc.tile_pool(name="obp", bufs=1) as obp, \
                     tc.tile_pool(name="smx", bufs=1) as smx, \
                     tc.tile_pool(name="outsb", bufs=4) as osb:
                    # single recip/acc/accb tiles shared across q-blocks
                    # (their lifetimes don't overlap). acc is the DVE-side
                    # softmax-sum accumulator (f32); accb its bf16 staging
                    # for the final cross-partition ones-matmul: moves the
                    # per-kb sum matmul (16384 cycles) off the PE
                    recip_t = smx.tile([P, QB], f32, name="recip")
                    acc_t = smx.tile([P, QB], f32, name="acc")
                    accb_t = smx.tile([P, QB], bf16, name="accb")
                    # Pool/SWDGE queue again: these generate after the kp
                    # stream + wq/qt0, landing well before first use (~160us+)
                    qt1 = qt1s.tile([P, DT, QB], bf16, name="qt1")
                    nc.gpsimd.dma_start(qt1[:], qT_r[:, :, QB:2 * QB])
                    ow_t = owp.tile([P, ET, D], bf16, name="ow_t")
                    nc.gpsimd.dma_start(ow_t[:, 0:4, :], ow_r[:, 0:4, :])
                    nc.gpsimd.dma_start(ow_t[:, 4:8, :], ow_r[:, 4:8, :])
                    ob_t = obp.tile([P, D], f32, name="ob_t")
                    nc.gpsimd.dma_start(ob_t[:], ob_bc)
                    obsel_t = obp.tile([P, P], bf16, name="obsel_t")
                    nc.gpsimd.dma_start(obsel_t[:], ob_sel)
                    obmat_t = obp.tile([P, D], bf16, name="obmat_t")
                    nc.gpsimd.dma_start(obmat_t[:], ob_mat)

                    def out_group(qb, g, spare_pool, store_eng=None):
                        """out[qb*QB+mq*128 : +128, nd*512 : +512] (8 mm)."""
                        nd, mq = divmod(g, MQ)
                        ps = spare_pool.tile([P, DNB], f32, tag="sp",
                                             name=f"ops{qb}_{g}")
                        for e in range(ET):
                            mm(ps[:], ctxs[qb][:, e, mq * P:(mq + 1) * P],
                               ow_t[:, e, nd * DNB:(nd + 1) * DNB],
                               e == 0, e == ET - 1)
                        r0 = qb * QB + mq * P
                        ot = osb.tile([P, DNB], bf16, tag="ot",
                                      name=f"ot{qb}_{g}")
                        nc.vector.tensor_add(
                            ot[:], ps[:], ob_t[:, nd * DNB:(nd + 1) * DNB])
                        (store_eng or nc.gpsimd).dma_start(
                            out[r0:r0 + P, nd * DNB:(nd + 1) * DNB], ot[:])

                    def out_group_tail(qb, g, tail_pool):
                        """Last group: bias folded in via the delta-row
                        matmul, evac by plain ACT copy (no DVE bias add on
                        the critical path), 2 column chunks so chunk 0's
                        store overlaps chunk 1's matmuls; final store rides
                        the fast HWDGE queue."""
                        nd, mq = divmod(g, MQ)
                        r0 = qb * QB + mq * P
                        engs = [nc.scalar, nc.sync]
                        widths = [3 * DNB // 4, DNB // 4]
                        for j in range(2):
                            c0 = nd * DNB + j * widths[0]
                            w = widths[j]
                            ps = tail_pool.tile([P, w], f32, tag=f"tp{j}",
                                                name=f"opst{qb}_{g}_{j}")
                            ot = osb.tile([P, w], bf16, tag=f"ott{j}",
                                          name=f"ott{qb}_{g}_{j}")
                            if j == 0:
                                # DVE bias-add evac so the two chunks'
                                # evacuations run on different engines
                                for e in range(ET):
                                    mm(ps[:],
                                       ctxs[qb][:, e, mq * P:(mq + 1) * P],
                                       ow_t[:, e, c0:c0 + w],
                                       e == 0, e == ET - 1)
                                nc.vector.tensor_add(
                                    ot[:], ps[:], ob_t[:, c0:c0 + w])
                            else:
                                mm(ps[:], obsel_t[:], obmat_t[:, c0:c0 + w],
                                   True, False)
                                for e in range(ET):
                                    mm(ps[:],
                                       ctxs[qb][:, e, mq * P:(mq + 1) * P],
                                       ow_t[:, e, c0:c0 + w],
                                       False, e == ET - 1)
                                nc.scalar.activation(ot[:], ps[:], AF.Identity)
                            engs[j].dma_start(
                                out[r0:r0 + P, c0:c0 + w], ot[:])

                    for qb in range(NQB):
                        # banks 0-3: ctx half1 accumulators (live whole block)
                        cps_cm = tc.tile_pool(name=f"cps{qb}", bufs=1,
                                              space="PSUM")
                        cpsp = cps_cm.__enter__()
                        cps = [cpsp.tile([P, QB], f32, name=f"c{qb}_{e}")
                               for e in range(ET // 2)]
                        # banks 4-7: spare + logits triple-buffer. Open order
                        # matters: the allocator hands the most-recently-freed
                        # banks to the first-opened pool, and the previous
                        # qb's last-freed banks (ctx-half2, evacuated latest)
                        # must NOT land on lg, whose first use is immediate.
                        # spare's first use (kb>=1) absorbs that latency.
                        with tc.tile_pool(name=f"spare{qb}", bufs=1,
                                          space="PSUM") as spp, \
                             tc.tile_pool(name=f"lg{qb}", bufs=3,
                                          space="PSUM") as lgp:

                            def lg_mm(kb):
                                ps = lgp.tile([P, QB], f32, tag="lg",
                                              name=f"lg{qb}_{kb}")
                                for e in range(ET):
                                    mm(ps[:], kp[:, e, kb * P:(kb + 1) * P],
                                       qps[qb][:, e, :], e == 0, e == ET - 1)
                                nc.scalar.activation(
                                    expT[:, kb, :], ps[:], AF.Exp,
                                    bias=mask_t[:, kb:kb + 1])
                                # fold exp tile into the DVE softmax-sum acc
                                if kb == 0:
                                    nc.vector.tensor_copy(acc_t[:],
                                                          expT[:, 0, :])
                                else:
                                    nc.vector.tensor_add(acc_t[:],
                                                         acc_t[:],
                                                         expT[:, kb, :])

                            def tail_mm(kb):
                                for e in range(ET // 2):
                                    mm(cps[e][:],
                                       vp[:, kb, e * P:(e + 1) * P],
                                       expT[:, kb, :], kb == 0, kb == KT - 1)

                            for kb in range(KT):
                                lg_mm(kb)
                                if kb > 0:
                                    tail_mm(kb - 1)
                                # spare-bank work, one group per slot:
                                if qb == 0 and kb >= ET:
                                    # qp(qb1) group m = kb-8
                                    m = kb - ET
                                    ps = spp.tile([P, QB], f32, tag="sp",
                                                  name=f"qps1_{m}")
                                    for t in range(DT):
                                        mm(ps[:], wq_t[:, t, m * P:(m + 1) * P],
                                           qt1[:, t, :], t == 0, t == DT - 1)
                                    nc.scalar.activation(
                                        qps[1][:, m, :], ps[:], AF.Identity,
                                        bias=bq_t[:, m:m + 1], scale=ISCALE)
                                if qb == 1 and kb % 2 == 1:
                                    out_group(0, kb // 2, spp)
                            tail_mm(KT - 1)

                        # ctx half2 on a 2-bank ping-pong: only two banks
                        # inherit late evacuations, so the next qb's logits
                        # pool (first-fit) lands on early-freed banks and
                        # starts without waiting. One more bank carries the
                        # cross-partition ones-matmul of the DVE sum acc.
                        with tc.tile_pool(name=f"sum{qb}", bufs=1,
                                          space="PSUM") as sump, \
                             tc.tile_pool(name=f"cps2_{qb}", bufs=2,
                                          space="PSUM") as cps2p:
                            nc.vector.tensor_copy(accb_t[:], acc_t[:])
                            s_ps = sump.tile([P, QB], f32, name=f"sps{qb}")
                            mm(s_ps[:], ones_t[:], accb_t[:],
                               True, True)
                            nc.vector.reciprocal(recip_t[:], s_ps[:])
                            # evacuate half1 (DVE) while half2 accumulates
                            for e in range(ET // 2):
                                nc.vector.tensor_mul(ctxs[qb][:, e, :],
                                                     cps[e][:],
                                                     recip_t[:])
                            for ei in range(ET // 2):
                                e = ET // 2 + ei
                                c2 = cps2p.tile([P, QB], f32, tag="c2",
                                                name=f"c2_{qb}_{e}")
                                for kb in range(KT):
                                    mm(c2[:],
                                       vp[:, kb, e * P:(e + 1) * P],
                                       expT[:, kb, :], kb == 0, kb == KT - 1)
                                # evac right away so the bank frees for the
                                # next sweep / next qb's pools
                                nc.vector.tensor_mul(ctxs[qb][:, e, :],
                                                     c2[:],
                                                     recip_t[:])
                        cps_cm.__exit__(None, None, None)

                    # ---- final out phase: out(qb1) ----
                    with tc.tile_pool(name="out_ps", bufs=2,
                                      space="PSUM") as ops, \
                         tc.tile_pool(name="tail_ps", bufs=1,
                                      space="PSUM") as tps:
                        # final-phase stores ride the idle HWDGE queues
                        # (scalar/sync) so Pool has nothing to drain at the
                        # kernel tail
                        for g in range(ND * MQ - 1):
                            out_group(1, g, ops,
                                      store_eng=(nc.sync, nc.scalar)[g % 2])
                        out_group_tail(1, ND * MQ - 1, tps)

    nc.compile()
    return nc


def make_in_maps(v, k, q, mask, wq_w, wq_b, wk_w, wk_b, wv_w, wv_b, out_w, out_b,
                 n_cores=8, D=1024, E=1024, SK=2048, QSH=1024):
    """Host-side shard + layout prep (data movement + bf16 cast, no math)."""
    import ml_dtypes
    bf = ml_dtypes.bfloat16
    ET = E // P
    KT = SK // P
    f = np.float32
    wq_w = np.ascontiguousarray(np.asarray(wq_w, f).astype(bf))
    wk_w = np.ascontiguousarray(np.asarray(wk_w, f).astype(bf))
    wv_w = np.ascontiguousarray(np.asarray(wv_w, f).astype(bf))
    out_w = np.ascontiguousarray(np.asarray(out_w, f).astype(bf))
    bq_col = np.ascontiguousarray(np.asarray(wq_b, f).reshape(ET, P).T)
    bk_col = np.ascontiguousarray(np.asarray(wk_b, f).reshape(ET, P).T)
    bv_bc = np.ascontiguousarray(
        np.broadcast_to(np.asarray(wv_b, f).astype(bf), (P, E)))
    ob_bc = np.ascontiguousarray(
        np.broadcast_to(np.asarray(out_b, f), (P, len(out_b))))
    ones_arr = np.ones((P, P), bf)
    ob_sel = np.zeros((P, P), bf)
    ob_sel[0, :] = 1
    ob_mat = np.zeros((P, len(out_b)), bf)
    ob_mat[0, :] = np.asarray(out_b, f).astype(bf)
    in_maps = []
    for c in range(n_cores):
        b, h = divmod(c, 2)
        qTc = np.ascontiguousarray(
            np.asarray(q[b, h * QSH:(h + 1) * QSH, :], f).T.astype(bf))
        kTc = np.ascontiguousarray(np.asarray(k[b], f).T.astype(bf))
        vTc = np.ascontiguousarray(np.asarray(v[b], f).T.astype(bf))
        mc = np.ascontiguousarray(np.asarray(mask[b, 0], f).reshape(KT, P).T)
        in_maps.append(dict(qT=qTc, kT=kTc, vT=vTc, mask_cols=mc,
                            ones_d=ones_arr,
                            wq=wq_w, wk=wk_w, wv=wv_w, ow=out_w,
                            bq_col=bq_col, bk_col=bk_col,
                            bv_bc=bv_bc, ob_bc=ob_bc,
                            ob_sel=ob_sel, ob_mat=ob_mat))
    return in_maps


_NC_CACHE = {}


def kernel(v, k, q, mask, wq_w, wq_b, wk_w, wk_b, wv_w, wv_b, out_w, out_b):
    from concourse.bass_utils import run_bass_kernel_spmd

    B, S, D = 4, 2048, 1024
    E, QSH = 1024, 1024
    if "nc" not in _NC_CACHE:
        _NC_CACHE["nc"] = build_nc(D=D, E=E, SK=S, QSH=QSH, QB=512)
    nc = _NC_CACHE["nc"]

    in_maps = make_in_maps(v, k, q, mask, wq_w, wq_b, wk_w, wk_b, wv_w, wv_b,
                           out_w, out_b, n_cores=8, D=D, E=E, SK=S, QSH=QSH)
    trace = bool(int(os.environ.get("BASS_KERNEL_TRACE", "0")))
    res = run_bass_kernel_spmd(nc, in_maps, core_ids=list(range(8)), trace=trace)
    if trace:
        print(f"HW exec time: {res.exec_time_ns} ns")
        _NC_CACHE["last_exec_time_ns"] = res.exec_time_ns
        _NC_CACHE["last_trace"] = res.instructions_and_trace

    outp = np.empty((B, S, D), np.float32)
    for c in range(8):
        b, h = divmod(c, 2)
        # stores are bf16 (halves store DMA bytes); upcast host-side
        outp[b, h * QSH:(h + 1) * QSH, :] = np.asarray(
            res.results[c]["out"]).astype(np.float32)
    return outp



# revision 31
# speedup vs baseline: 1.0062x; 1.0028x over previous
"""Single-head attention (B=4, S=2048, D=E=1024) on 8 trn2 NeuronCores.

Sharding: data-parallel over (batch, q-half) -> 8 shards. Each core gets a
1024-row q shard plus the full 2048 keys of its batch; K/V projections are
recomputed on both cores of a batch pair (25% extra flops, zero collectives).

All matmul operands are bf16 (host-cast); PSUM accumulation stays fp32, so
per-value RMS error ~0.1% -- far inside the 2e-2 gate. bf16 runs at the same
1 cycle/row PE rate as fp32r but halves DMA + SBUF, which lets every weight
stay resident (no DRAM bounce) and keeps the PE streaming continuously:

  per-core PE work (cycles @2.4GHz):
    vp 131072 + kp 131072 + qp 65536 + logits 131072 + ctx 131072
    + out 65536 + sum-reduce 1024 = 656k cycles = 273.5us ideal
  (the per-kb softmax-sum ones-matmuls run on DVE as f32 adds instead,
   leaving PE only one 128-partition ones-reduce per q-block)

Schedule: vp -> kp -> qp(qb0) -> qb0 kb-loop [logits|exp|ctx-half1 on PE,
sum-acc on DVE, qp(qb1) in the spare PSUM bank] -> ctx-half2 -> qb1 kb-loop
[with out(qb0) in the spare bank] -> ctx-half2 -> out(qb1, stores on the
idle HWDGE queues; tail chunks evacuate on DVE||ACT in parallel). PSUM never
exceeds 8 banks; weights for each phase are prefetched during the previous
phase via sibling pools.
"""

import os
import numpy as np

P = 128
NEG = -1.0e9


def build_nc(D=1024, E=1024, SK=2048, QSH=1024, QB=512):
    """Build the per-core Bass module (SPMD; same program on all cores)."""
    import concourse.bass as bass
    import concourse.mybir as mybir
    import concourse.tile as tile
    from concourse import bacc

    f32 = mybir.dt.float32
    bf16 = mybir.dt.bfloat16
    AF = mybir.ActivationFunctionType

    DT = D // P          # contraction tiles over model dim
    ET = E // P          # enc tiles
    KT = SK // P         # key tiles
    NQB = QSH // QB      # q blocks (2)
    KNB = 512            # key free-dim block for kp
    DNB = 512            # model free-dim block for out
    MQ = QB // P         # q 128-row groups per block (4)
    ND = D // DNB        # out column chunks (2)
    ISCALE = 1.0 / float(np.sqrt(E))

    nc = bacc.Bacc(trn_type="TRN2")

    # ---- I/O (bf16 operands; f32 biases/mask; bf16 output,
    # upcast host-side -- halves store-DMA bytes) ----
    qT = nc.dram_tensor("qT", [D, QSH], bf16, kind="ExternalInput")[:, :]
    kT = nc.dram_tensor("kT", [D, SK], bf16, kind="ExternalInput")[:, :]
    vT = nc.dram_tensor("vT", [D, SK], bf16, kind="ExternalInput")[:, :]
    mask_cols = nc.dram_tensor("mask_cols", [P, KT], f32, kind="ExternalInput")[:, :]
    ones_d = nc.dram_tensor("ones_d", [P, P], bf16, kind="ExternalInput")[:, :]
    wq = nc.dram_tensor("wq", [D, E], bf16, kind="ExternalInput")[:, :]
    wk = nc.dram_tensor("wk", [D, E], bf16, kind="ExternalInput")[:, :]
    wv = nc.dram_tensor("wv", [D, E], bf16, kind="ExternalInput")[:, :]
    ow = nc.dram_tensor("ow", [E, D], bf16, kind="ExternalInput")[:, :]
    bq_col = nc.dram_tensor("bq_col", [P, ET], f32, kind="ExternalInput")[:, :]
    bk_col = nc.dram_tensor("bk_col", [P, ET], f32, kind="ExternalInput")[:, :]
    bv_bc = nc.dram_tensor("bv_bc", [P, E], bf16, kind="ExternalInput")[:, :]
    ob_bc = nc.dram_tensor("ob_bc", [P, D], f32, kind="ExternalInput")[:, :]
    # delta-row selector + ob with row 0 = out bias: the final out group
    # folds its bias in via matmul so the evac is a plain ACT copy
    ob_sel = nc.dram_tensor("ob_sel", [P, P], bf16, kind="ExternalInput")[:, :]
    ob_mat = nc.dram_tensor("ob_mat", [P, D], bf16, kind="ExternalInput")[:, :]
    out = nc.dram_tensor("out", [QSH, D], bf16, kind="ExternalOutput")[:, :]

    qT_r = qT.rearrange("(t p) n -> p t n", p=P)   # [128, DT, QSH]
    kT_r = kT.rearrange("(t p) n -> p t n", p=P)
    vT_r = vT.rearrange("(t p) n -> p t n", p=P)
    wq_r = wq.rearrange("(t p) n -> p t n", p=P)   # [128, DT, E]
    wk_r = wk.rearrange("(t p) n -> p t n", p=P)
    wv_r = wv.rearrange("(t p) n -> p t n", p=P)
    ow_r = ow.rearrange("(t p) n -> p t n", p=P)   # [128, ET, D]

    def mm(ps, lhsT, rhs, start, stop):
        nc.tensor.matmul(ps, lhsT, rhs, start=start, stop=stop)

    NWARM = 8

    with tile.TileContext(nc) as tc:
        # ---- persistent smalls (tiles allocated here; DMAs emitted inside
        # the AB scope so the scalar queue prioritizes wv chunks) ----
        with tc.tile_pool(name="smalls", bufs=1) as smalls:
            bv_t = smalls.tile([P, E], bf16, name="bv_t")
            mask_t = smalls.tile([P, KT], f32, name="maskc")
            bk_t = smalls.tile([P, ET], f32, name="bkc")
            bq_t = smalls.tile([P, ET], f32, name="bqc")
            ones_t = smalls.tile([P, P], bf16, name="ones")

            # persistent operand tensors
            with tc.tile_pool(name="wqp", bufs=1) as wqp, \
                 tc.tile_pool(name="vpp", bufs=1) as vpp, \
                 tc.tile_pool(name="kpp", bufs=1) as kpp, \
                 tc.tile_pool(name="qpp", bufs=1) as qpp, \
                 tc.tile_pool(name="expp", bufs=1) as expp, \
                 tc.tile_pool(name="ctxp", bufs=1) as ctxp:
                wq_t = wqp.tile([P, DT, E], bf16, name="wq_t")
                vp = vpp.tile([P, KT, E], bf16, name="vp")      # [k, E]
                kp = kpp.tile([P, ET, SK], bf16, name="kp")     # [E, k] (kp^T)
                qps = [qpp.tile([P, ET, QB], bf16, name=f"qp{i}")
                       for i in range(NQB)]                      # [E, q] (qp^T)
                expT = expp.tile([P, KT, QB], bf16, name="expT")  # [k, q]
                ctxs = [ctxp.tile([P, ET, QB], bf16, name=f"ctx{i}")
                        for i in range(NQB)]                     # [E, q] (ctx^T)

                # ============ phase A+B: vp then kp (sibling pools so kp
                # weights prefetch during vp) ============
                with tc.tile_pool(name="wv_w", bufs=1) as wvp, \
                     tc.tile_pool(name="wk_w", bufs=1) as wkp, \
                     tc.tile_pool(name="vT_s", bufs=2) as vts, \
                     tc.tile_pool(name="kT_s", bufs=2) as kts, \
                     tc.tile_pool(name="qT0_s", bufs=1) as qt0s, \
                     tc.tile_pool(name="warm", bufs=1) as warm, \
                     tc.tile_pool(name="warm_ps", bufs=1,
                                  space="PSUM") as wps, \
                     tc.tile_pool(name="ab_ps", bufs=3, space="PSUM") as abps:
                    # PE warm-up: dummy matmuls on a memset tile fill the
                    # ~5.7us wait for the first weight DMA and ramp the PE
                    # p-state so real work starts at full clock. Lives in
                    # the AB scope so nothing aliases (and WAR-waits on) it.
                    junk = warm.tile([P, 512], bf16, name="junk")
                    nc.vector.memset(junk[:], 0.0)
                    wp = wps.tile([P, 512], f32, name="warmps")
                    for i in range(NWARM):
                        nc.tensor.matmul(wp[:], junk[:, 0:P], junk[:],
                                         start=(i == 0),
                                         stop=(i == NWARM - 1))
                    # The shared DMA bus serves transfers in descriptor-gen
                    # completion order, so every queue is sequenced by first
                    # NEED: sync gets only the first wv half; Pool (slow 1.3us
                    # SWDGE gen each = natural pacing) carries the whole
                    # vp/kp-phase stream in consumption order; scalar gets the
                    # smalls then the late-needed wq/qt0.
                    wv_t = wvp.tile([P, DT, E], bf16, name="wv_t")
                    wk_t = wkp.tile([P, DT, E], bf16, name="wk_t")
                    nc.sync.dma_start(wv_t[:, 0:4, 0:512], wv_r[:, 0:4, 0:512])
                    nc.scalar.dma_start(wv_t[:, 4:8, 0:512],
                                        wv_r[:, 4:8, 0:512])
                    qt0 = qt0s.tile([P, DT, QB], bf16, name="qt0")

                    # -- vp: psum [128k, 512E] per (m, n) group; vT streamed
                    # in 1MB chunks of 4 k-tiles (desc count is per (p,t),
                    # so wider chunks halve Pool SWDGE time) --
                    vtiles = {}

                    def load_vt(c):
                        vt = vts.tile([P, DT, 4 * P], bf16, tag="vt",
                                      name=f"vt{c}")
                        if c == 0:
                            # four [4t x 256col] pieces (512B runs) whose
                            # descs interleave with the wv quarters on the
                            # serial bus, enabling the two-pass t-split
                            # start below
                            nc.gpsimd.dma_start(vt[:, 0:4, 0:2 * P],
                                                vT_r[:, 0:4, 0:2 * P])
                            nc.gpsimd.dma_start(vt[:, 4:8, 0:2 * P],
                                                vT_r[:, 4:8, 0:2 * P])
                            nc.gpsimd.dma_start(vt[:, 0:4, 2 * P:4 * P],
                                                vT_r[:, 0:4, 2 * P:4 * P])
                            nc.gpsimd.dma_start(vt[:, 4:8, 2 * P:4 * P],
                                                vT_r[:, 4:8, 2 * P:4 * P])
                        else:
                            nc.gpsimd.dma_start(
                                vt[:], vT_r[:, :, 4 * c * P:(4 * c + 4) * P])
                        vtiles[c] = vt

                    def vp_group(m, n):
                        ps = abps.tile([P, 512], f32, tag="ps",
                                       name=f"vps{m}_{n}")
                        vt = vtiles[m // 4]
                        mi = m % 4
                        for t in range(DT):
                            mm(ps[:], vt[:, t, mi * P:(mi + 1) * P],
                               wv_t[:, t, n * 512:(n + 1) * 512],
                               t == 0, t == DT - 1)
                        nc.vector.tensor_add(
                            vp[:, m, n * 512:(n + 1) * 512], ps[:],
                            bv_t[:, n * 512:(n + 1) * 512])

                    # chunk 0: n=0 groups first (wv col half 1 still loading)
                    load_vt(0)

                    def vp_pass(ps, m, n, t0, t1):
                        if ps is None:
                            ps = abps.tile([P, 512], f32, tag="ps",
                                           name=f"vps2_{m}_{n}")
                        vt = vtiles[m // 4]
                        mi = m % 4
                        for t in range(t0, t1):
                            mm(ps[:], vt[:, t, mi * P:(mi + 1) * P],
                               wv_t[:, t, n * 512:(n + 1) * 512],
                               t == 0, t == DT - 1)
                        return ps
                    # Pool/SWDGE queue carries everything else in strict
                    # consumption order (scalar/sync are HWDGE-fast and would
                    # let late-needed weights jump the shared bus)
                    nc.gpsimd.dma_start(bv_t[:, 0:512], bv_bc[:, 0:512])
                    nc.gpsimd.dma_start(wv_t[:, :, 512:1024],
                                        wv_r[:, :, 512:1024])
                    nc.gpsimd.dma_start(bv_t[:, 512:1024], bv_bc[:, 512:1024])
                    load_vt(1)
                    psA = {}
                    for m in range(2):
                        psA[m] = vp_pass(None, m, 0, 0, 4)
                    for m in range(2):
                        ps = psA.pop(m)
                        vp_pass(ps, m, 0, 4, DT)
                        nc.vector.tensor_add(
                            vp[:, m, 0:512], ps[:], bv_t[:, 0:512])
                    for m in range(2, 4):
                        vp_group(m, 0)
                    for m in range(4):
                        vp_group(m, 1)
                    for c in range(1, KT // 4):
                        if c + 1 < KT // 4:
                            load_vt(c + 1)
                        for mi in range(4):
                            for n in range(E // 512):
                                vp_group(4 * c + mi, n)
                    # smalls on scalar, emitted after the vp stream so their
                    # descs land on the bus behind the critical wv/vt0 set
                    # (first use: bk ~35us, bq ~75us, mask ~90us, ones ~180us)
                    nc.scalar.dma_start(bk_t[:], bk_col)
                    nc.scalar.dma_start(mask_t[:], mask_cols)
                    nc.scalar.mul(mask_t[:], mask_t[:], NEG)
                    nc.scalar.dma_start(bq_t[:], bq_col)
                    nc.scalar.mul(bq_t[:], bq_t[:], ISCALE)
                    nc.scalar.dma_start(ones_t[:], ones_d)
                    # wk lands on the bus after vt2/vt3 (needed at kp start)
                    nc.gpsimd.dma_start(wk_t[:, :, 0:512], wk_r[:, :, 0:512])
                    nc.gpsimd.dma_start(wk_t[:, :, 512:1024],
                                        wk_r[:, :, 512:1024])

                    # -- kp: for each k-chunk, psum [128E, 512k] x8 --
                    for n in range(SK // KNB):
                        kt = kts.tile([P, DT, KNB], bf16, tag="kt",
                                      name=f"kt{n}")
                        nc.gpsimd.dma_start(kt[:],
                                            kT_r[:, :, n * KNB:(n + 1) * KNB])
                        for m in range(ET):
                            ps = abps.tile([P, KNB], f32, tag="ps",
                                           name=f"kps{n}_{m}")
                            for t in range(DT):
                                mm(ps[:], wk_t[:, t, m * P:(m + 1) * P],
                                   kt[:, t, :], t == 0, t == DT - 1)
                            nc.scalar.activation(
                                kp[:, m, n * KNB:(n + 1) * KNB], ps[:],
                                AF.Identity, bias=bk_t[:, m:m + 1])

                    # wq/qt0 queue behind the kp stream; kt2/kt3's WAR head-
                    # of-line block paces their generation to ~90us, arriving
                    # in time for qp0 at ~116us
                    nc.gpsimd.dma_start(wq_t[:, 0:4, :], wq_r[:, 0:4, :])
                    nc.gpsimd.dma_start(wq_t[:, 4:8, :], wq_r[:, 4:8, :])
                    nc.gpsimd.dma_start(qt0[:], qT_r[:, :, 0:QB])

                    # -- qp(qb0): psum [128E, 512q] x8 (reuse ab psum bufs) --
                    for m in range(ET):
                        ps = abps.tile([P, QB], f32, tag="ps", name=f"qps0_{m}")
                        for t in range(DT):
                            mm(ps[:], wq_t[:, t, m * P:(m + 1) * P],
                               qt0[:, t, :], t == 0, t == DT - 1)
                        nc.scalar.activation(qps[0][:, m, :], ps[:],
                                             AF.Identity,
                                             bias=bq_t[:, m:m + 1],
                                             scale=ISCALE)

                # ============ attention (ow/qT1/out-staging reuse AB space) ==
                with tc.tile_pool(name="ow_w", bufs=1) as owp, \
                     tc.tile_pool(name="qT1_s", bufs=1) as qt1s, \
                     tc.tile_pool(name="obp", bufs=1) as obp, \
                     tc.tile_pool(name="smx", bufs=1) as smx, \
                     tc.tile_pool(name="outsb", bufs=4) as osb:
                    # single recip/acc/accb tiles shared across q-blocks
                    # (their lifetimes don't overlap). acc is the DVE-side
                    # softmax-sum accumulator (f32); accb its bf16 staging
                    # for the final cross-partition ones-matmul: moves the
                    # per-kb sum matmul (16384 cycles) off the PE
                    recip_t = smx.tile([P, QB], f32, name="recip")
                    acc_t = smx.tile([P, QB], f32, name="acc")
                    accb_t = smx.tile([P, QB], bf16, name="accb")
                    # Pool/SWDGE queue again: these generate after the kp
                    # stream + wq/qt0, landing well before first use (~160us+)
                    qt1 = qt1s.tile([P, DT, QB], bf16, name="qt1")
                    nc.gpsimd.dma_start(qt1[:], qT_r[:, :, QB:2 * QB])
                    ow_t = owp.tile([P, ET, D], bf16, name="ow_t")
                    nc.gpsimd.dma_start(ow_t[:, 0:4, :], ow_r[:, 0:4, :])
                    nc.gpsimd.dma_start(ow_t[:, 4:8, :], ow_r[:, 4:8, :])
                    ob_t = obp.tile([P, D], f32, name="ob_t")
                    nc.gpsimd.dma_start(ob_t[:], ob_bc)
                    obsel_t = obp.tile([P, P], bf16, name="obsel_t")
                    nc.gpsimd.dma_start(obsel_t[:], ob_sel)
                    obmat_t = obp.tile([P, D], bf16, name="obmat_t")
                    nc.gpsimd.dma_start(obmat_t[:], ob_mat)

                    def out_group(qb, g, spare_pool, store_eng=None):
                        """out[qb*QB+mq*128 : +128, nd*512 : +512] (8 mm)."""
                        nd, mq = divmod(g, MQ)
                        ps = spare_pool.tile([P, DNB], f32, tag="sp",
                                             name=f"ops{qb}_{g}")
                        for e in range(ET):
                            mm(ps[:], ctxs[qb][:, e, mq * P:(mq + 1) * P],
                               ow_t[:, e, nd * DNB:(nd + 1) * DNB],
                               e == 0, e == ET - 1)
                        r0 = qb * QB + mq * P
                        ot = osb.tile([P, DNB], bf16, tag="ot",
                                      name=f"ot{qb}_{g}")
                        nc.vector.tensor_add(
                            ot[:], ps[:], ob_t[:, nd * DNB:(nd + 1) * DNB])
                        (store_eng or nc.gpsimd).dma_start(
                            out[r0:r0 + P, nd * DNB:(nd + 1) * DNB], ot[:])

                    def out_group_tail(qb, g, tail_pool):
                        """Last group: bias folded in via the delta-row
                        matmul, evac by plain ACT copy (no DVE bias add on
                        the critical path), 2 column chunks so chunk 0's
                        store overlaps chunk 1's matmuls; final store rides
                        the fast HWDGE queue."""
                        nd, mq = divmod(g, MQ)
                        r0 = qb * QB + mq * P
                        engs = [nc.scalar, nc.sync]
                        widths = [3 * DNB // 4, DNB // 4]
                        for j in range(2):
                            c0 = nd * DNB + j * widths[0]
                            w = widths[j]
                            ps = tail_pool.tile([P, w], f32, tag=f"tp{j}",
                                                name=f"opst{qb}_{g}_{j}")
                            ot = osb.tile([P, w], bf16, tag=f"ott{j}",
                                          name=f"ott{qb}_{g}_{j}")
                            if j == 0:
                                # DVE bias-add evac so the two chunks'
                                # evacuations run on different engines
                                for e in range(ET):
                                    mm(ps[:],
                                       ctxs[qb][:, e, mq * P:(mq + 1) * P],
                                       ow_t[:, e, c0:c0 + w],
                                       e == 0, e == ET - 1)
                                nc.vector.tensor_add(
                                    ot[:], ps[:], ob_t[:, c0:c0 + w])
                            else:
                                mm(ps[:], obsel_t[:], obmat_t[:, c0:c0 + w],
                                   True, False)
                                for e in range(ET):
                                    mm(ps[:],
                                       ctxs[qb][:, e, mq * P:(mq + 1) * P],
                                       ow_t[:, e, c0:c0 + w],
                                       False, e == ET - 1)
                                nc.scalar.activation(ot[:], ps[:], AF.Identity)
                            engs[j].dma_start(
                                out[r0:r0 + P, c0:c0 + w], ot[:])

                    for qb in range(NQB):
                        # banks 0-3: ctx half1 accumulators (live whole block)
                        cps_cm = tc.tile_pool(name=f"cps{qb}", bufs=1,
                                              space="PSUM")
                        cpsp = cps_cm.__enter__()
                        cps = [cpsp.tile([P, QB], f32, name=f"c{qb}_{e}")
                               for e in range(ET // 2)]
                        # banks 4-7: spare + logits triple-buffer. Open order
                        # matters: the allocator hands the most-recently-freed
                        # banks to the first-opened pool, and the previous
                        # qb's last-freed banks (ctx-half2, evacuated latest)
                        # must NOT land on lg, whose first use is immediate.
                        # spare's first use (kb>=1) absorbs that latency.
                        with tc.tile_pool(name=f"spare{qb}", bufs=1,
                                          space="PSUM") as spp, \
                             tc.tile_pool(name=f"lg{qb}", bufs=3,
                                          space="PSUM") as lgp:

                            def lg_mm(kb):
                                ps = lgp.tile([P, QB], f32, tag="lg",
                                              name=f"lg{qb}_{kb}")
                                for e in range(ET):
                                    mm(ps[:], kp[:, e, kb * P:(kb + 1) * P],
                                       qps[qb][:, e, :], e == 0, e == ET - 1)
                                nc.scalar.activation(
                                    expT[:, kb, :], ps[:], AF.Exp,
                                    bias=mask_t[:, kb:kb + 1])
                                # fold exp tile into the DVE softmax-sum acc
                                if kb == 0:
                                    nc.vector.tensor_copy(acc_t[:],
                                                          expT[:, 0, :])
                                else:
                                    nc.vector.tensor_add(acc_t[:],
                                                         acc_t[:],
                                                         expT[:, kb, :])

                            def tail_mm(kb):
                                for e in range(ET // 2):
                                    mm(cps[e][:],
                                       vp[:, kb, e * P:(e + 1) * P],
                                       expT[:, kb, :], kb == 0, kb == KT - 1)

                            for kb in range(KT):
                                lg_mm(kb)
                                if kb > 0:
                                    tail_mm(kb - 1)
                                # spare-bank work, one group per slot:
                                if qb == 0 and kb >= ET:
                                    # qp(qb1) group m = kb-8
                                    m = kb - ET
                                    ps = spp.tile([P, QB], f32, tag="sp",
                                                  name=f"qps1_{m}")
                                    for t in range(DT):
                                        mm(ps[:], wq_t[:, t, m * P:(m + 1) * P],
                                           qt1[:, t, :], t == 0, t == DT - 1)
                                    nc.scalar.activation(
                                        qps[1][:, m, :], ps[:], AF.Identity,
                                        bias=bq_t[:, m:m + 1], scale=ISCALE)
                                if qb == 1 and kb % 2 == 1:
                                    out_group(0, kb // 2, spp)
                            tail_mm(KT - 1)

                        # ctx half2 on a 2-bank ping-pong: only two banks
                        # inherit late evacuations, so the next qb's logits
                        # pool (first-fit) lands on early-freed banks and
                        # starts without waiting. One more bank carries the
                        # cross-partition ones-matmul of the DVE sum acc.
                        with tc.tile_pool(name=f"sum{qb}", bufs=1,
                                          space="PSUM") as sump, \
                             tc.tile_pool(name=f"cps2_{qb}", bufs=2,
                                          space="PSUM") as cps2p:
                            nc.vector.tensor_copy(accb_t[:], acc_t[:])
                            s_ps = sump.tile([P, QB], f32, name=f"sps{qb}")
                            mm(s_ps[:], ones_t[:], accb_t[:],
                               True, True)
                            nc.vector.reciprocal(recip_t[:], s_ps[:])
                            # evacuate half1 (DVE) while half2 accumulates
                            for e in range(ET // 2):
                                nc.vector.tensor_mul(ctxs[qb][:, e, :],
                                                     cps[e][:],
                                                     recip_t[:])
                            for ei in range(ET // 2):
                                e = ET // 2 + ei
                                c2 = cps2p.tile([P, QB], f32, tag="c2",
                                                name=f"c2_{qb}_{e}")
                                for kb in range(KT):
                                    mm(c2[:],
                                       vp[:, kb, e * P:(e + 1) * P],
                                       expT[:, kb, :], kb == 0, kb == KT - 1)
                                # evac right away so the bank frees for the
                                # next sweep / next qb's pools
                                nc.vector.tensor_mul(ctxs[qb][:, e, :],
                                                     c2[:],
                                                     recip_t[:])
                        cps_cm.__exit__(None, None, None)

                    # ---- final out phase: out(qb1) ----
                    with tc.tile_pool(name="out_ps", bufs=2,
                                      space="PSUM") as ops, \
                         tc.tile_pool(name="tail_ps", bufs=1,
                                      space="PSUM") as tps:
                        # final-phase stores ride the idle HWDGE queues
                        # (scalar/sync) so Pool has nothing to drain at the
                        # kernel tail
                        for g in range(ND * MQ - 1):
                            out_group(1, g, ops,
                                      store_eng=(nc.sync, nc.scalar)[g % 2])
                        out_group_tail(1, ND * MQ - 1, tps)

    nc.compile()
    return nc


def make_in_maps(v, k, q, mask, wq_w, wq_b, wk_w, wk_b, wv_w, wv_b, out_w, out_b,
                 n_cores=8, D=1024, E=1024, SK=2048, QSH=1024):
    """Host-side shard + layout prep (data movement + bf16 cast, no math)."""
    import ml_dtypes
    bf = ml_dtypes.bfloat16
    ET = E // P
    KT = SK // P
    f = np.float32
    wq_w = np.ascontiguousarray(np.asarray(wq_w, f).astype(bf))
    wk_w = np.ascontiguousarray(np.asarray(wk_w, f).astype(bf))
    wv_w = np.ascontiguousarray(np.asarray(wv_w, f).astype(bf))
    out_w = np.ascontiguousarray(np.asarray(out_w, f).astype(bf))
    bq_col = np.ascontiguousarray(np.asarray(wq_b, f).reshape(ET, P).T)
    bk_col = np.ascontiguousarray(np.asarray(wk_b, f).reshape(ET, P).T)
    bv_bc = np.ascontiguousarray(
        np.broadcast_to(np.asarray(wv_b, f).astype(bf), (P, E)))
    ob_bc = np.ascontiguousarray(
        np.broadcast_to(np.asarray(out_b, f), (P, len(out_b))))
    ones_arr = np.ones((P, P), bf)
    ob_sel = np.zeros((P, P), bf)
    ob_sel[0, :] = 1
    ob_mat = np.zeros((P, len(out_b)), bf)
    ob_mat[0, :] = np.asarray(out_b, f).astype(bf)
    in_maps = []
    for c in range(n_cores):
        b, h = divmod(c, 2)
        qTc = np.ascontiguousarray(
            np.asarray(q[b, h * QSH:(h + 1) * QSH, :], f).T.astype(bf))
        kTc = np.ascontiguousarray(np.asarray(k[b], f).T.astype(bf))
        vTc = np.ascontiguousarray(np.asarray(v[b], f).T.astype(bf))
        mc = np.ascontiguousarray(np.asarray(mask[b, 0], f).reshape(KT, P).T)
        in_maps.append(dict(qT=qTc, kT=kTc, vT=vTc, mask_cols=mc,
                            ones_d=ones_arr,
                            wq=wq_w, wk=wk_w, wv=wv_w, ow=out_w,
                            bq_col=bq_col, bk_col=bk_col,
                            bv_bc=bv_bc, ob_bc=ob_bc,
                            ob_sel=ob_sel, ob_mat=ob_mat))
    return in_maps


_NC_CACHE = {}


def kernel(v, k, q, mask, wq_w, wq_b, wk_w, wk_b, wv_w, wv_b, out_w, out_b):
    from concourse.bass_utils import run_bass_kernel_spmd

    B, S, D = 4, 2048, 1024
    E, QSH = 1024, 1024
    if "nc" not in _NC_CACHE:
        _NC_CACHE["nc"] = build_nc(D=D, E=E, SK=S, QSH=QSH, QB=512)
    nc = _NC_CACHE["nc"]

    in_maps = make_in_maps(v, k, q, mask, wq_w, wq_b, wk_w, wk_b, wv_w, wv_b,
                           out_w, out_b, n_cores=8, D=D, E=E, SK=S, QSH=QSH)
    trace = bool(int(os.environ.get("BASS_KERNEL_TRACE", "0")))
    res = run_bass_kernel_spmd(nc, in_maps, core_ids=list(range(8)), trace=trace)
    if trace:
        print(f"HW exec time: {res.exec_time_ns} ns")
        _NC_CACHE["last_exec_time_ns"] = res.exec_time_ns
        _NC_CACHE["last_trace"] = res.instructions_and_trace

    outp = np.empty((B, S, D), np.float32)
    for c in range(8):
        b, h = divmod(c, 2)
        # stores are bf16 (halves store DMA bytes); upcast host-side
        outp[b, h * QSH:(h + 1) * QSH, :] = np.asarray(
            res.results[c]["out"]).astype(np.float32)
    return outp

